# revision 16
# baseline (speedup 1.0000x reference)
"""Batched GCN (microtubule dynamics model) on 8 Trainium2 NeuronCores.

Math: the reference's gather/scale/scatter message passing over a fixed
52-node graph is a dense linear operator on the node axis:
    agg[b] = A @ h[b],  A[i, j] = sum over edges (j->i, incl self-loops)
                                   of dinv[src] * dinv[dst]
and A commutes with the shared linear layer, so each GNN layer is
    x += relu((A @ x) @ W_l^T + b_l),  batched over B.

Device strategy (pure data parallel, 512 batch elems / core):
  - activations live in SBUF as [128 hid partitions, 26624 token cols]
    (token = b*52 + node), fp16 on-chip, fp32 PSUM for matmuls
  - per layer, per macro group of 8 batch elems (832 token cols):
      8x fused transpose+W matmuls (stationary = 104-token X window,
      moving = W_l^T) into a 2-bank PSUM tile -> ONE PSUM->SBUF copy
      (1024 cols) -> 8x node-mix matmuls (stationary = h^T chunk,
      moving = blockdiag(A^T,A^T)) into a second 2-bank PSUM tile
      (416-col runs at bank-aligned offsets) -> ONE fused relu+bias
      (strided 2x416 AP) into an 8-group staging tile -> one
      accumulating SWDGE DMA per 8 groups does the residual
  - PSUM-reading vector work cannot run on Pool (BIR: GPSIMD cannot
    access PSUM), so copies and relu+bias round-robin over DVE and ACT
    at a 4:5 ratio (inverse of their cycle times); the Pool engine only
    triggers the SWDGE residual DMAs
  - decoder: d1 like a layer; d2 exploits that matmul cost ~ moving
    free size: stationary = relu(d1) 104-token chunk, moving = W_d2^T
    (6 cols) -> token-major y in PSUM, copied once per 64 chunks and
    DMA'd to HBM as [TOK, 6] fp16; b_d2 is added during host unshard
"""

import numpy as np

N_FIL, N_SUB = 13, 4
N_NODES = N_FIL * N_SUB          # 52
FEAT = 6
HID = 128
N_LAYERS = 3
BATCH = 4096
N_CORES = 8
B_C = BATCH // N_CORES           # 512 batch elems per core
TOK = B_C * N_NODES              # 26624 token columns per core
PAIR_T = 2 * N_NODES             # 104 tokens per transpose chunk
MAC_T = 8 * PAIR_T               # 832 token columns per macro group
N_MACRO = TOK // MAC_T           # 32 macro groups per layer
RES_M = 2                        # macros per residual accum DMA (the
                                 # SWDGE accumulate path corrupts data
                                 # when a per-partition contiguous run
                                 # exceeds ~4KB, so keep runs at 3328B)
SLICE = 512                      # psum bank (fp32 cols)
YCHUNKS = 64                     # d2 token-chunks per psum y tile

_CACHE = {}
_LAST_EXEC_NS = None
_LAST_TRACE = []
_DEBUG = False   # adds dbgX0/dbgX1 DRAM outputs (X after enc / after layer 1)
_KVER = 5        # bump on every semantic change: the execution service caches
                 # compiled NEFFs by program signature, and a changing input
                 # shape is the only reliable cache-buster



def _build_nc():
    import concourse.bacc as bacc
    import concourse.mybir as mybir
    from concourse.tile import TileContext

    f32 = mybir.dt.float32
    f16 = mybir.dt.float16
    Alu = mybir.AluOpType
    Relu = mybir.ActivationFunctionType.Relu

    nc = bacc.Bacc(trn_type="TRN2")

    # blob16 cols: [0:384] wgT (3 layers x 128), [384:512] wd1T,
    # [512:518] wd2T, [518:622] a2 (rows 0:104)
    qT_d = nc.dram_tensor("qT", [FEAT, TOK], f16, kind="ExternalInput")
    winT_d = nc.dram_tensor("winT", [FEAT, HID], f16, kind="ExternalInput")
    blob_d = nc.dram_tensor("blob16", [HID, 622], f16, kind="ExternalInput")
    # bias cols: 0 b_in, 1..3 b_gnn, 4 b_d1
    bias_d = nc.dram_tensor("biases", [HID, 5], f32, kind="ExternalInput")
    ver_d = nc.dram_tensor("vertag", [1, _KVER], f32, kind="ExternalInput")
    y_d = nc.dram_tensor("yTm", [TOK, FEAT], f16, kind="ExternalOutput")
    if _DEBUG:
        dbg0_d = nc.dram_tensor("dbgX0", [HID, TOK], f16, kind="ExternalOutput")
        dbg1_d = nc.dram_tensor("dbgX1", [HID, TOK], f16, kind="ExternalOutput")

    # greedy DVE/ACT balance: assign each PSUM-exit op to the engine with
    # the lower projected busy time (cost-model rates incl. fixed overheads)
    busy = {"A": 0.0, "D": 0.0}

    def next_eng(cols):
        ca = cols * 0.8333 + 215.0
        cd = cols * 1.0417 + 140.0
        if busy["A"] + ca / 2 <= busy["D"] + cd / 2:
            busy["A"] += ca
            return "A"
        busy["D"] += cd
        return "D"

    def opa(out, psum, bias_ap, zero):
        # out = relu(psum + bias) on DVE or ACT
        if next_eng(out.shape[-1]) == "A":
            nc.scalar.activation(out, psum, Relu, bias=bias_ap)
        else:
            nc.vector.scalar_tensor_tensor(
                out, psum, bias_ap, zero, op0=Alu.add, op1=Alu.max
            )

    def copy(out, psum):
        if next_eng(out.shape[-1]) == "A":
            nc.scalar.copy(out, psum)
        else:
            nc.vector.tensor_copy(out, psum)

    with TileContext(nc) as tc:
        with (
            tc.sbuf_pool(name="const", bufs=1) as cp,
            tc.sbuf_pool(name="work", bufs=4) as wp,
            tc.psum_pool(name="ps", bufs=2) as pp,
        ):
            blob = cp.tile_from(blob_d[:, :])
            winT = cp.tile_from(winT_d[:, :])
            biases = cp.tile_from(bias_d[:, :])
            zero_s = cp.tile([128, 1024], f16)
            nc.vector.memset(zero_s, 0.0)

            wd1T = blob[:, 384:512]
            wd2T = blob[:, 512:518]
            a2 = blob[:PAIR_T, 518:622]
            bin_s = biases[:, 0:1]
            bd1_s = biases[:, 4:5]

            qT = cp.tile([FEAT, TOK], f16)
            nc.sync.dma_start(qT, qT_d[:, :])
            vtag = cp.tile([1, _KVER], f32)
            nc.sync.dma_start(vtag, ver_d[:, :])

            X = cp.tile([128, TOK], f16)

            # -------- encoder: X = relu(W_in @ q^T + b_in) ----------------
            for t in range(TOK // 1024):            # 26 tiles of 2 slices
                enc_ps = pp.tile([128, 1024], f32, tag="ps_c", bufs=2)
                for q in range(2):
                    nc.tensor.matmul(
                        enc_ps[:, q * SLICE:(q + 1) * SLICE],
                        winT, qT[:, t * 1024 + q * SLICE:
                                 t * 1024 + (q + 1) * SLICE],
                        start=True, stop=True,
                    )
                opa(X[:, t * 1024:(t + 1) * 1024], enc_ps, bin_s, zero_s)

            if _DEBUG:
                nc.sync.dma_start(dbg0_d[:, :], X)

            # -------- GNN layers: x += relu(A (x W_l^T) + b_l) -----------
            # software-pipelined: MM1 for macro m+1 is emitted BEFORE the
            # node-mix of macro m, so the in-order PE queue never stalls
            # waiting for the PSUM->SBUF copy of macro m
            for l in range(N_LAYERS):
                wgT_l = blob[:, l * 128:(l + 1) * 128]
                bg_l = biases[:, 1 + l:2 + l]
                ht_tiles = {}
                for m in range(N_MACRO + 1):
                    if m < N_MACRO:
                        # fused transpose+W: 8 x (stationary = X 104-token
                        # window, moving = W_l^T) -> h^T (token-major)
                        c0 = m * MAC_T
                        ht_ps = pp.tile([128, 1024], f32, tag="ps_b", bufs=2)
                        for p in range(8):
                            nc.tensor.matmul(
                                ht_ps[:PAIR_T, p * 128:(p + 1) * 128],
                                X[:, c0 + p * PAIR_T:c0 + (p + 1) * PAIR_T],
                                wgT_l,
                                start=True, stop=True,
                            )
                        ht_tiles[m] = ht_ps
                    if m == 0:
                        continue
                    mm = m - 1
                    ht_ps = ht_tiles.pop(mm)
                    hts = wp.tile([128, 1024], f16, bufs=4)
                    copy(hts[:PAIR_T, :], ht_ps[:PAIR_T, :])

                    # node mix back to hid-major; 416-col runs at the two
                    # bank-aligned offsets (cols 416..511 unused)
                    agg_ps = pp.tile([128, 1024], f32, tag="ps_c", bufs=2)
                    for p in range(8):
                        g, q = divmod(p, 4)
                        nc.tensor.matmul(
                            agg_ps[:, g * SLICE + q * PAIR_T:
                                      g * SLICE + (q + 1) * PAIR_T],
                            hts[:PAIR_T, p * 128:(p + 1) * 128],
                            a2,
                            start=True, stop=True,
                        )

                    # relu+bias into a 2-macro staging tile; one accumulating
                    # SWDGE DMA per 2 macros does the residual (SWDGE
                    # accumulate corrupts beyond ~4KB contiguous runs)
                    if mm % RES_M == 0:
                        r8 = wp.tile([128, RES_M * MAC_T], f16, bufs=3,
                                     name="r8")
                    off = (mm % RES_M) * MAC_T
                    for g in range(2):
                        opa(
                            r8[:, off + g * 416:off + (g + 1) * 416],
                            agg_ps[:, g * SLICE:g * SLICE + 416],
                            bg_l, zero_s[:, :416],
                        )
                    if mm % RES_M == RES_M - 1:
                        nc.gpsimd.dma_start(
                            X[:, (mm - RES_M + 1) * MAC_T:(mm + 1) * MAC_T],
                            r8, accum_op=Alu.add,
                        )
                if _DEBUG and l == 0:
                    nc.sync.dma_start(dbg1_d[:, :], X)

            # -------- decoder --------------------------------------------
            # d1 slices are batch-agnostic, so use full 512-col matmuls and
            # 1024-col relu+bias into one persistent d1s tile; d2 chunks
            # (104 tokens) then slice d1s at pair boundaries
            d1s = cp.tile([128, TOK], f16)
            for t in range(TOK // 1024):
                d1_ps = pp.tile([128, 1024], f32, tag="ps_c", bufs=2)
                for q in range(2):
                    nc.tensor.matmul(
                        d1_ps[:, q * SLICE:(q + 1) * SLICE],
                        wd1T,
                        X[:, t * 1024 + q * SLICE:t * 1024 + (q + 1) * SLICE],
                        start=True, stop=True,
                    )
                opa(d1s[:, t * 1024:(t + 1) * 1024], d1_ps, bd1_s, zero_s)

            # y chunk = (W_d2 @ relu(d1))^T: stationary = d1s 104-token
            # chunk, moving = W_d2^T (6 cols) -> token-major y
            for m in range(N_MACRO):
                if m % 8 == 0:
                    y_ps = pp.tile([PAIR_T, YCHUNKS * FEAT], f32,
                                   tag="ps_b", bufs=2)
                for p in range(8):
                    j = (m % 8) * 8 + p
                    c = m * 8 + p
                    nc.tensor.matmul(
                        y_ps[:, j * FEAT:(j + 1) * FEAT],
                        d1s[:, c * PAIR_T:(c + 1) * PAIR_T],
                        wd2T,
                        start=True, stop=True,
                    )
                if m % 8 == 7:
                    k = m // 8
                    y16 = wp.tile([PAIR_T, YCHUNKS * FEAT], f16, bufs=2)
                    nc.vector.tensor_copy(y16, y_ps)
                    dst = y_d[k * YCHUNKS * PAIR_T:
                              (k + 1) * YCHUNKS * PAIR_T, :]
                    nc.sync.dma_start(
                        dst.rearrange("(c t) f -> t c f", t=PAIR_T),
                        y16.rearrange("t (c f) -> t c f", f=FEAT),
                    )

    nc.finalize()
    return nc


def _host_prep(inputs):
    q = np.asarray(inputs["q_current"], np.float32).reshape(BATCH, N_NODES, FEAT)
    W_in = np.asarray(inputs["W_in"], np.float32)
    b_in = np.asarray(inputs["b_in"], np.float32)
    W_gnn = np.asarray(inputs["W_gnn"], np.float32)
    b_gnn = np.asarray(inputs["b_gnn"], np.float32)
    W_d1 = np.asarray(inputs["W_d1"], np.float32)
    b_d1 = np.asarray(inputs["b_d1"], np.float32)
    W_d2 = np.asarray(inputs["W_d2"], np.float32)
    edge = np.asarray(inputs["edge_index"]).astype(np.int64)

    # dense normalized adjacency (PyG GCNConv default w/ self-loops)
    loops = np.arange(N_NODES, dtype=np.int64)
    src = np.concatenate([edge[0], loops])
    dst = np.concatenate([edge[1], loops])
    deg = np.zeros(N_NODES, np.float32)
    np.add.at(deg, dst, 1.0)
    dinv = 1.0 / np.sqrt(np.maximum(deg, 1e-12))
    A = np.zeros((N_NODES, N_NODES), np.float32)
    np.add.at(A, (dst, src), dinv[src] * dinv[dst])

    a2 = np.zeros((PAIR_T, PAIR_T), np.float32)
    a2[:N_NODES, :N_NODES] = A.T
    a2[N_NODES:, N_NODES:] = A.T

    blob = np.zeros((HID, 622), np.float32)
    blob[:, 0:384] = W_gnn.transpose(2, 0, 1).reshape(HID, N_LAYERS * HID)
    blob[:, 384:512] = W_d1.T
    blob[:, 512:518] = W_d2.T
    blob[:PAIR_T, 518:622] = a2

    biases = np.zeros((HID, 5), np.float32)
    biases[:, 0] = b_in
    biases[:, 1:4] = b_gnn.T
    biases[:, 4] = b_d1

    const = {
        "winT": np.ascontiguousarray(W_in.T).astype(np.float16),
        "blob16": blob.astype(np.float16),
        "biases": np.ascontiguousarray(biases),
    }

    # per-core feature-major input [6, TOK], fp16
    q_flat = q.reshape(N_CORES, B_C * N_NODES, FEAT)
    qTs = [
        np.ascontiguousarray(q_flat[c].T).astype(np.float16)
        for c in range(N_CORES)
    ]
    return const, qTs


def kernel(**inputs):
    const, qTs = _host_prep(inputs)

    if "nc" not in _CACHE:
        _CACHE["nc"] = _build_nc()
    nc = _CACHE["nc"]

    from concourse.bass_utils import run_bass_kernel_spmd

    const["vertag"] = np.zeros((1, _KVER), np.float32)
    in_maps = [dict(const, qT=qTs[c]) for c in range(N_CORES)]
    res = run_bass_kernel_spmd(nc, in_maps, core_ids=list(range(N_CORES)))
    global _LAST_EXEC_NS
    _LAST_EXEC_NS = res.exec_time_ns
    if res.instructions_and_trace is not None:
        _LAST_TRACE.append(res.instructions_and_trace[1])

    b_d2 = np.asarray(inputs["b_d2"], np.float32)
    outs = []
    for c in range(N_CORES):
        yTm = res.results[c]["yTm"]  # [TOK, 6] fp16
        outs.append(np.asarray(yTm, np.float32) + b_d2)
    y = np.concatenate(outs, axis=0)  # [BATCH*52, 6]
    return np.ascontiguousarray(y).reshape(BATCH, N_FIL, N_SUB, FEAT)


# revision 20
# speedup vs baseline: 1.1286x; 1.1286x over previous
"""Batched GCN (microtubule dynamics model) on 8 Trainium2 NeuronCores.

Math: the reference's gather/scale/scatter message passing over a fixed
52-node graph is a dense linear operator on the node axis:
    agg[b] = A @ h[b],  A[i, j] = sum over edges (j->i, incl self-loops)
                                   of dinv[src] * dinv[dst]
and A commutes with the shared linear layer, so each GNN layer is
    x += relu((A @ x) @ W_l^T + b_l),  batched over B.

Device strategy (pure data parallel, 512 batch elems / core):
  - activations live in SBUF as [128 hid partitions, 26624 token cols]
    (token = b*52 + node), fp16 on-chip, fp32 PSUM for matmuls
  - per layer, per macro group of 8 batch elems (832 token cols):
      8x fused transpose+W matmuls (stationary = 104-token X window,
      moving = W_l^T) into a 2-bank PSUM tile -> ONE PSUM->SBUF copy
      (1024 cols) -> 8x node-mix matmuls (stationary = h^T chunk,
      moving = blockdiag(A^T,A^T)) into a second 2-bank PSUM tile
      (416-col runs at bank-aligned offsets) -> ONE fused relu+bias
      (strided 2x416 AP) into an 8-group staging tile -> one
      accumulating SWDGE DMA per 8 groups does the residual
  - PSUM-reading vector work cannot run on Pool (BIR: GPSIMD cannot
    access PSUM), so copies and relu+bias round-robin over DVE and ACT
    at a 4:5 ratio (inverse of their cycle times); the Pool engine only
    triggers the SWDGE residual DMAs
  - decoder: d1 like a layer; d2 exploits that matmul cost ~ moving
    free size: stationary = relu(d1) 104-token chunk, moving = W_d2^T
    (6 cols) -> token-major y in PSUM, copied once per 64 chunks and
    DMA'd to HBM as [TOK, 6] fp16; b_d2 is added during host unshard
"""

import numpy as np

N_FIL, N_SUB = 13, 4
N_NODES = N_FIL * N_SUB          # 52
FEAT = 6
HID = 128
N_LAYERS = 3
BATCH = 4096
N_CORES = 8
B_C = BATCH // N_CORES           # 512 batch elems per core
TOK = B_C * N_NODES              # 26624 token columns per core
PAIR_T = 2 * N_NODES             # 104 tokens per transpose chunk
MAC_T = 8 * PAIR_T               # 832 token columns per macro group
N_MACRO = TOK // MAC_T           # 32 macro groups per layer
RES_M = 2                        # macros per residual accum DMA (the
                                 # SWDGE accumulate path corrupts data
                                 # when a per-partition contiguous run
                                 # exceeds ~4KB, so keep runs at 3328B)
SLICE = 512                      # psum bank (fp32 cols)
N_SLICES = TOK // SLICE          # 52
YCHUNKS = 64                     # d2 token-chunks per psum y tile

_CACHE = {}
_LAST_EXEC_NS = None
_LAST_TRACE = []
_DEBUG = False   # adds dbgX0/dbgX1 DRAM outputs (X after enc / after layer 1)
_KVER = 6        # bump on every semantic change: the execution service caches
                 # compiled NEFFs by program signature, and a changing input
                 # shape is the only reliable cache-buster



def _build_nc():
    import concourse.bacc as bacc
    import concourse.mybir as mybir
    from concourse.tile import TileContext

    f32 = mybir.dt.float32
    f16 = mybir.dt.float16
    Alu = mybir.AluOpType
    Relu = mybir.ActivationFunctionType.Relu

    nc = bacc.Bacc(trn_type="TRN2")

    # blob16 cols: [0:384] wgT (3 layers x 128), [384:512] wd1T,
    # [512:518] wd2T, [518:622] a2 (rows 0:104)
    qT_d = nc.dram_tensor("qT", [FEAT, TOK], f16, kind="ExternalInput")
    winT_d = nc.dram_tensor("winT", [FEAT, HID], f16, kind="ExternalInput")
    blob_d = nc.dram_tensor("blob16", [HID, 622], f16, kind="ExternalInput")
    # bias cols: 0 b_in, 1..3 b_gnn, 4 b_d1
    bias_d = nc.dram_tensor("biases", [HID, 5], f32, kind="ExternalInput")
    ver_d = nc.dram_tensor("vertag", [1, _KVER], f32, kind="ExternalInput")
    y_d = nc.dram_tensor("yTm", [TOK, FEAT], f16, kind="ExternalOutput")
    if _DEBUG:
        dbg0_d = nc.dram_tensor("dbgX0", [HID, TOK], f16, kind="ExternalOutput")
        dbg1_d = nc.dram_tensor("dbgX1", [HID, TOK], f16, kind="ExternalOutput")

    # greedy DVE/ACT balance: assign each PSUM-exit op to the engine with
    # the lower projected busy time (cost-model rates incl. fixed overheads)
    busy = {"A": 0.0, "D": 0.0}

    def next_eng(cols):
        ca = cols * 0.8333 + 215.0
        cd = cols * 1.0417 + 140.0
        if busy["A"] + ca / 2 <= busy["D"] + cd / 2:
            busy["A"] += ca
            return "A"
        busy["D"] += cd
        return "D"

    def opa(out, psum, bias_ap, zero):
        # out = relu(psum + bias) on DVE or ACT
        if next_eng(out.shape[-1]) == "A":
            nc.scalar.activation(out, psum, Relu, bias=bias_ap)
        else:
            nc.vector.scalar_tensor_tensor(
                out, psum, bias_ap, zero, op0=Alu.add, op1=Alu.max
            )

    def copy(out, psum):
        if next_eng(out.shape[-1]) == "A":
            nc.scalar.copy(out, psum)
        else:
            nc.vector.tensor_copy(out, psum)

    with TileContext(nc) as tc:
        with (
            tc.sbuf_pool(name="const", bufs=1) as cp,
            tc.sbuf_pool(name="work", bufs=4) as wp,
            tc.psum_pool(name="ps", bufs=2) as pp,
        ):
            blob = cp.tile_from(blob_d[:, :])
            winT = cp.tile_from(winT_d[:, :])
            biases = cp.tile_from(bias_d[:, :])
            zero_s = cp.tile([128, 1024], f16)
            nc.vector.memset(zero_s, 0.0)

            wd1T = blob[:, 384:512]
            wd2T = blob[:, 512:518]
            a2 = blob[:PAIR_T, 518:622]
            bin_s = biases[:, 0:1]
            bd1_s = biases[:, 4:5]

            qT = cp.tile([FEAT, TOK], f16)
            nc.sync.dma_start(qT, qT_d[:, :])
            vtag = cp.tile([1, _KVER], f32)
            nc.sync.dma_start(vtag, ver_d[:, :])

            X = cp.tile([128, TOK], f16)

            # -------- encoder: X = relu(W_in @ q^T + b_in) ----------------
            for s_ in range(N_SLICES):
                enc_ps = pp.tile([128, SLICE], f32,
                                 tag="pc" if s_ % 2 else "pb", bufs=4,
                                 name="encps")
                nc.tensor.matmul(
                    enc_ps, winT,
                    qT[:, s_ * SLICE:(s_ + 1) * SLICE],
                    start=True, stop=True,
                )
                opa(X[:, s_ * SLICE:(s_ + 1) * SLICE], enc_ps, bin_s,
                    zero_s[:, :SLICE])

            if _DEBUG:
                nc.sync.dma_start(dbg0_d[:, :], X)

            # -------- GNN layers: x += relu(A (x W_l^T) + b_l) -----------
            # software-pipelined: MM1 for macro m+1 is emitted BEFORE the
            # node-mix of macro m, so the in-order PE queue never stalls
            # waiting for the PSUM->SBUF copy of macro m
            for l in range(N_LAYERS):
                wgT_l = blob[:, l * 128:(l + 1) * 128]
                bg_l = biases[:, 1 + l:2 + l]
                ht_tiles = {}
                for m in range(N_MACRO + 1):
                    if m < N_MACRO:
                        # fused transpose+W: 8 x (stationary = X 104-token
                        # window, moving = W_l^T) -> h^T (token-major)
                        c0 = m * MAC_T
                        ht_ps = [
                            pp.tile([128, SLICE], f32, tag="pb", bufs=4,
                                    name="htq")
                            for _ in range(2)
                        ]
                        for p in range(8):
                            nc.tensor.matmul(
                                ht_ps[p // 4][:PAIR_T,
                                              (p % 4) * 128:(p % 4 + 1) * 128],
                                X[:, c0 + p * PAIR_T:c0 + (p + 1) * PAIR_T],
                                wgT_l,
                                start=True, stop=True,
                            )
                        ht_tiles[m] = ht_ps
                    if m == 0:
                        continue
                    mm = m - 1
                    ht_ps = ht_tiles.pop(mm)
                    hts = wp.tile([128, 1024], f16, bufs=6)
                    for hh in range(2):
                        copy(hts[:PAIR_T, hh * SLICE:(hh + 1) * SLICE],
                             ht_ps[hh][:PAIR_T, :])

                    # node mix back to hid-major, one 1-bank tile per group
                    agg_ps = [
                        pp.tile([128, SLICE], f32, tag="pc", bufs=4,
                                name="aggq")
                        for _ in range(2)
                    ]
                    for p in range(8):
                        g, q = divmod(p, 4)
                        nc.tensor.matmul(
                            agg_ps[g][:, q * PAIR_T:(q + 1) * PAIR_T],
                            hts[:PAIR_T, p * 128:(p + 1) * 128],
                            a2,
                            start=True, stop=True,
                        )

                    # relu+bias into a 2-macro staging tile; one accumulating
                    # SWDGE DMA per 2 macros does the residual (SWDGE
                    # accumulate corrupts beyond ~4KB contiguous runs)
                    if mm % RES_M == 0:
                        r8 = wp.tile([128, RES_M * MAC_T], f16, bufs=3,
                                     name="r8")
                    off = (mm % RES_M) * MAC_T
                    for g in range(2):
                        opa(
                            r8[:, off + g * 416:off + (g + 1) * 416],
                            agg_ps[g][:, :416],
                            bg_l, zero_s[:, :416],
                        )
                    if mm % RES_M == RES_M - 1:
                        nc.gpsimd.dma_start(
                            X[:, (mm - RES_M + 1) * MAC_T:(mm + 1) * MAC_T],
                            r8, accum_op=Alu.add,
                        )
                if _DEBUG and l == 0:
                    nc.sync.dma_start(dbg1_d[:, :], X)

            # -------- decoder --------------------------------------------
            # d1 slices are batch-agnostic, so use full 512-col matmuls and
            # 1024-col relu+bias into one persistent d1s tile; d2 chunks
            # (104 tokens) then slice d1s at pair boundaries
            d1s = cp.tile([128, TOK], f16)
            for s_ in range(N_SLICES):
                d1_ps = pp.tile([128, SLICE], f32,
                                tag="pc" if s_ % 2 else "pb", bufs=4,
                                name="d1ps")
                nc.tensor.matmul(
                    d1_ps, wd1T,
                    X[:, s_ * SLICE:(s_ + 1) * SLICE],
                    start=True, stop=True,
                )
                opa(d1s[:, s_ * SLICE:(s_ + 1) * SLICE], d1_ps, bd1_s,
                    zero_s[:, :SLICE])

            # y chunk = (W_d2 @ relu(d1))^T: stationary = d1s 104-token
            # chunk, moving = W_d2^T (6 cols) -> token-major y
            for m in range(N_MACRO):
                if m % 8 == 0:
                    y_ps = pp.tile([PAIR_T, YCHUNKS * FEAT], f32,
                                   tag="pb", bufs=4)
                for p in range(8):
                    j = (m % 8) * 8 + p
                    c = m * 8 + p
                    nc.tensor.matmul(
                        y_ps[:, j * FEAT:(j + 1) * FEAT],
                        d1s[:, c * PAIR_T:(c + 1) * PAIR_T],
                        wd2T,
                        start=True, stop=True,
                    )
                if m % 8 == 7:
                    k = m // 8
                    y16 = wp.tile([PAIR_T, YCHUNKS * FEAT], f16, bufs=2)
                    nc.vector.tensor_copy(y16, y_ps)
                    dst = y_d[k * YCHUNKS * PAIR_T:
                              (k + 1) * YCHUNKS * PAIR_T, :]
                    nc.sync.dma_start(
                        dst.rearrange("(c t) f -> t c f", t=PAIR_T),
                        y16.rearrange("t (c f) -> t c f", f=FEAT),
                    )

    nc.finalize()
    return nc


def _host_prep(inputs):
    q = np.asarray(inputs["q_current"], np.float32).reshape(BATCH, N_NODES, FEAT)
    W_in = np.asarray(inputs["W_in"], np.float32)
    b_in = np.asarray(inputs["b_in"], np.float32)
    W_gnn = np.asarray(inputs["W_gnn"], np.float32)
    b_gnn = np.asarray(inputs["b_gnn"], np.float32)
    W_d1 = np.asarray(inputs["W_d1"], np.float32)
    b_d1 = np.asarray(inputs["b_d1"], np.float32)
    W_d2 = np.asarray(inputs["W_d2"], np.float32)
    edge = np.asarray(inputs["edge_index"]).astype(np.int64)

    # dense normalized adjacency (PyG GCNConv default w/ self-loops)
    loops = np.arange(N_NODES, dtype=np.int64)
    src = np.concatenate([edge[0], loops])
    dst = np.concatenate([edge[1], loops])
    deg = np.zeros(N_NODES, np.float32)
    np.add.at(deg, dst, 1.0)
    dinv = 1.0 / np.sqrt(np.maximum(deg, 1e-12))
    A = np.zeros((N_NODES, N_NODES), np.float32)
    np.add.at(A, (dst, src), dinv[src] * dinv[dst])

    a2 = np.zeros((PAIR_T, PAIR_T), np.float32)
    a2[:N_NODES, :N_NODES] = A.T
    a2[N_NODES:, N_NODES:] = A.T

    blob = np.zeros((HID, 622), np.float32)
    blob[:, 0:384] = W_gnn.transpose(2, 0, 1).reshape(HID, N_LAYERS * HID)
    blob[:, 384:512] = W_d1.T
    blob[:, 512:518] = W_d2.T
    blob[:PAIR_T, 518:622] = a2

    biases = np.zeros((HID, 5), np.float32)
    biases[:, 0] = b_in
    biases[:, 1:4] = b_gnn.T
    biases[:, 4] = b_d1

    const = {
        "winT": np.ascontiguousarray(W_in.T).astype(np.float16),
        "blob16": blob.astype(np.float16),
        "biases": np.ascontiguousarray(biases),
    }

    # per-core feature-major input [6, TOK], fp16
    q_flat = q.reshape(N_CORES, B_C * N_NODES, FEAT)
    qTs = [
        np.ascontiguousarray(q_flat[c].T).astype(np.float16)
        for c in range(N_CORES)
    ]
    return const, qTs


def kernel(**inputs):
    const, qTs = _host_prep(inputs)

    if "nc" not in _CACHE:
        _CACHE["nc"] = _build_nc()
    nc = _CACHE["nc"]

    from concourse.bass_utils import run_bass_kernel_spmd

    const["vertag"] = np.zeros((1, _KVER), np.float32)
    in_maps = [dict(const, qT=qTs[c]) for c in range(N_CORES)]
    res = run_bass_kernel_spmd(nc, in_maps, core_ids=list(range(N_CORES)))
    global _LAST_EXEC_NS
    _LAST_EXEC_NS = res.exec_time_ns
    if res.instructions_and_trace is not None:
        _LAST_TRACE.append(res.instructions_and_trace[1])

    b_d2 = np.asarray(inputs["b_d2"], np.float32)
    outs = []
    for c in range(N_CORES):
        yTm = res.results[c]["yTm"]  # [TOK, 6] fp16
        outs.append(np.asarray(yTm, np.float32) + b_d2)
    y = np.concatenate(outs, axis=0)  # [BATCH*52, 6]
    return np.ascontiguousarray(y).reshape(BATCH, N_FIL, N_SUB, FEAT)


# revision 21
# speedup vs baseline: 1.1890x; 1.0536x over previous
"""Batched GCN (microtubule dynamics model) on 8 Trainium2 NeuronCores.

Math: the reference's gather/scale/scatter message passing over a fixed
52-node graph is a dense linear operator on the node axis:
    agg[b] = A @ h[b],  A[i, j] = sum over edges (j->i, incl self-loops)
                                   of dinv[src] * dinv[dst]
and A commutes with the shared linear layer, so each GNN layer is
    x += relu((A @ x) @ W_l^T + b_l),  batched over B.

Device strategy (pure data parallel, 512 batch elems / core):
  - activations live in SBUF as [128 hid partitions, 26624 token cols]
    (token = b*52 + node), fp16 on-chip, fp32 PSUM for matmuls
  - per layer the token axis is processed in FULL 128-token windows
    (no per-batch-pair padding): fused transpose+W matmuls (stationary
    = 128-token X window, moving = W_l^T) fill one PSUM bank per 4
    windows -> ONE PSUM->SBUF copy per 512 tokens -> the node mix uses
    13 precomputed phase matrices A_phi[src 128, 4*52] (phi = window
    offset mod 52) as the moving operand, accumulating into per-bank
    agg PSUM tiles with start/stop groups -> relu+bias per full 512-col
    bank -> one accumulating SWDGE DMA per 4 banks does the residual
    (the SWDGE accumulate path corrupts beyond 4KB contiguous runs, so
    runs are kept at exactly 4KB)
  - PSUM-reading vector work cannot run on Pool (BIR: GPSIMD cannot
    access PSUM), so copies and relu+bias are greedily balanced over
    DVE and ACT by projected busy time; Pool only triggers SWDGE
  - decoder: d1 like the encoder; d2 exploits that matmul cost ~ moving
    free size: stationary = relu(d1) 104-token chunk, moving = W_d2^T
    (6 cols) -> token-major y in PSUM, copied once per 64 chunks and
    DMA'd to HBM as [TOK, 6] fp16; b_d2 is added during host unshard
"""

import numpy as np

N_FIL, N_SUB = 13, 4
N_NODES = N_FIL * N_SUB          # 52
FEAT = 6
HID = 128
N_LAYERS = 3
BATCH = 4096
N_CORES = 8
B_C = BATCH // N_CORES           # 512 batch elems per core
TOK = B_C * N_NODES              # 26624 token columns per core
PAIR_T = 2 * N_NODES             # 104 tokens per decoder d2 chunk
WIN = 128                        # tokens per transpose window
N_WIN = TOK // WIN               # 208 windows per layer
SLICE = 512                      # psum bank (fp32 cols)
N_SLICES = TOK // SLICE          # 52 banks of tokens
RB = 4                           # agg banks per residual accum DMA (4KB)
YCHUNKS = 64                     # d2 token-chunks per psum y tile

_CACHE = {}
_LAST_EXEC_NS = None
_LAST_TRACE = []
_DEBUG = False   # adds dbgX0/dbgX1 DRAM outputs (X after enc / after layer 1)
_KVER = 7        # bump on every semantic change: the execution service caches
                 # compiled NEFFs by program signature, and a changing input
                 # shape is the only reliable cache-buster


def _phase_plan():
    """Per-window node-mix plan against 512-token agg banks."""
    phis = [(WIN * w) % N_NODES for w in range(N_WIN)]
    uniq = sorted(set(phis))                      # 13 phases
    pidx = {p: i for i, p in enumerate(uniq)}
    plan = []                                     # w -> [(bank, olo, ohi, alo, ahi)]
    contrib = {}                                  # bank -> [w, ...]
    for w in range(N_WIN):
        e0 = (WIN * w) // N_NODES
        t_lo = N_NODES * e0
        t_hi = min(t_lo + 4 * N_NODES, TOK)
        ent = []
        for b in range(t_lo // SLICE, (t_hi - 1) // SLICE + 1):
            lo = max(t_lo, SLICE * b)
            hi = min(t_hi, SLICE * (b + 1))
            ent.append((b, lo - SLICE * b, hi - SLICE * b, lo - t_lo, hi - t_lo))
            contrib.setdefault(b, []).append(w)
        plan.append(ent)
    return phis, uniq, pidx, plan, contrib


def _build_nc():
    import concourse.bacc as bacc
    import concourse.mybir as mybir
    from concourse.tile import TileContext

    f32 = mybir.dt.float32
    f16 = mybir.dt.float16
    Alu = mybir.AluOpType
    Relu = mybir.ActivationFunctionType.Relu

    nc = bacc.Bacc(trn_type="TRN2")

    phis, uniq, pidx, plan, contrib = _phase_plan()
    n_ph = len(uniq)
    APH0 = 518                    # a2ph block starts here in the blob

    # blob16 cols: [0:384] wgT (3 layers x 128), [384:512] wd1T,
    # [512:518] wd2T, [518:518+13*208] A_phi phase matrices
    qT_d = nc.dram_tensor("qT", [FEAT, TOK], f16, kind="ExternalInput")
    winT_d = nc.dram_tensor("winT", [FEAT, HID], f16, kind="ExternalInput")
    blob_d = nc.dram_tensor("blob16", [HID, APH0 + n_ph * 208], f16,
                            kind="ExternalInput")
    # bias cols: 0 b_in, 1..3 b_gnn, 4 b_d1
    bias_d = nc.dram_tensor("biases", [HID, 5], f32, kind="ExternalInput")
    ver_d = nc.dram_tensor("vertag", [1, _KVER], f32, kind="ExternalInput")
    y_d = nc.dram_tensor("yTm", [TOK, FEAT], f16, kind="ExternalOutput")
    if _DEBUG:
        dbg0_d = nc.dram_tensor("dbgX0", [HID, TOK], f16, kind="ExternalOutput")
        dbg1_d = nc.dram_tensor("dbgX1", [HID, TOK], f16, kind="ExternalOutput")

    # greedy DVE/ACT balance: assign each PSUM-exit op to the engine with
    # the lower projected busy time (cost-model rates incl. fixed overheads)
    busy = {"A": 0.0, "D": 0.0}

    def next_eng(cols):
        ca = cols * 0.8333 + 215.0
        cd = cols * 1.0417 + 140.0
        if busy["A"] + ca / 2 <= busy["D"] + cd / 2:
            busy["A"] += ca
            return "A"
        busy["D"] += cd
        return "D"

    def opa(out, psum, bias_ap, zero):
        # out = relu(psum + bias) on DVE or ACT
        if next_eng(out.shape[-1]) == "A":
            nc.scalar.activation(out, psum, Relu, bias=bias_ap)
        else:
            nc.vector.scalar_tensor_tensor(
                out, psum, bias_ap, zero, op0=Alu.add, op1=Alu.max
            )

    def copy(out, psum):
        if next_eng(out.shape[-1]) == "A":
            nc.scalar.copy(out, psum)
        else:
            nc.vector.tensor_copy(out, psum)

    with TileContext(nc) as tc:
        with (
            tc.sbuf_pool(name="const", bufs=1) as cp,
            tc.sbuf_pool(name="work", bufs=4) as wp,
            tc.psum_pool(name="ps", bufs=2) as pp,
        ):
            blob = cp.tile_from(blob_d[:, :])
            winT = cp.tile_from(winT_d[:, :])
            biases = cp.tile_from(bias_d[:, :])
            vtag = cp.tile([1, _KVER], f32)
            nc.sync.dma_start(vtag, ver_d[:, :])
            zero_s = cp.tile([128, SLICE], f16)
            nc.vector.memset(zero_s, 0.0)

            wd1T = blob[:, 384:512]
            wd2T = blob[:, 512:518]
            bin_s = biases[:, 0:1]
            bd1_s = biases[:, 4:5]

            qT = cp.tile([FEAT, TOK], f16)
            nc.sync.dma_start(qT, qT_d[:, :])

            X = cp.tile([128, TOK], f16)

            # -------- encoder: X = relu(W_in @ q^T + b_in) ----------------
            for s_ in range(N_SLICES):
                enc_ps = pp.tile([128, SLICE], f32,
                                 tag="pc" if s_ % 2 else "pb", bufs=4,
                                 name="encps")
                nc.tensor.matmul(
                    enc_ps, winT,
                    qT[:, s_ * SLICE:(s_ + 1) * SLICE],
                    start=True, stop=True,
                )
                opa(X[:, s_ * SLICE:(s_ + 1) * SLICE], enc_ps, bin_s, zero_s)

            if _DEBUG:
                nc.sync.dma_start(dbg0_d[:, :], X)

            # -------- GNN layers: x += relu(A (x W_l^T) + b_l) -----------
            # software-pipelined: the transpose+W matmuls for tile T are
            # emitted before the node mix of tile T-1 so the in-order PE
            # queue never stalls on the PSUM->SBUF copy
            for l in range(N_LAYERS):
                wgT_l = blob[:, l * 128:(l + 1) * 128]
                bg_l = biases[:, 1 + l:2 + l]
                ht_tiles = {}
                agg_open = {}
                r8 = None
                for T in range(N_SLICES + 1):
                    if T < N_SLICES:
                        ht_ps = pp.tile([128, SLICE], f32, tag="pb", bufs=4,
                                        name="htps")
                        for i in range(4):
                            w = 4 * T + i
                            nc.tensor.matmul(
                                ht_ps[:, i * WIN:(i + 1) * WIN],
                                X[:, w * WIN:(w + 1) * WIN],
                                wgT_l,
                                start=True, stop=True,
                            )
                        ht_tiles[T] = ht_ps
                    if T == 0:
                        continue
                    TT = T - 1
                    hts = wp.tile([128, SLICE], f16, bufs=6)
                    copy(hts, ht_tiles.pop(TT))

                    # node mix: moving = A_phi slices, accumulating into
                    # per-bank agg tiles (start on first / stop on last
                    # contribution)
                    for i in range(4):
                        w = 4 * TT + i
                        aph = blob[:, APH0 + pidx[phis[w]] * 208:
                                      APH0 + (pidx[phis[w]] + 1) * 208]
                        for (b, olo, ohi, alo, ahi) in plan[w]:
                            if b not in agg_open:
                                agg_open[b] = pp.tile(
                                    [128, SLICE], f32, tag="pc", bufs=4,
                                    name="aggps",
                                )
                            nc.tensor.matmul(
                                agg_open[b][:, olo:ohi],
                                hts[:, i * WIN:(i + 1) * WIN],
                                aph[:, alo:ahi],
                                start=(w == contrib[b][0]),
                                stop=(w == contrib[b][-1]),
                            )

                    # banks whose last contribution was in this tile:
                    # relu+bias into the residual staging tile, then one
                    # accumulating SWDGE DMA per RB banks (4KB runs)
                    for b in sorted(agg_open):
                        if contrib[b][-1] > 4 * TT + 3:
                            continue
                        if b % RB == 0:
                            r8 = wp.tile([128, RB * SLICE], f16, bufs=3,
                                         name="r8")
                        opa(r8[:, (b % RB) * SLICE:(b % RB + 1) * SLICE],
                            agg_open.pop(b), bg_l, zero_s)
                        if b % RB == RB - 1:
                            nc.gpsimd.dma_start(
                                X[:, (b - RB + 1) * SLICE:(b + 1) * SLICE],
                                r8, accum_op=Alu.add,
                            )
                    assert len(agg_open) <= 3, (T, sorted(agg_open))
                if _DEBUG and l == 0:
                    nc.sync.dma_start(dbg1_d[:, :], X)

            # -------- decoder --------------------------------------------
            # d1 slices are batch-agnostic; d2 chunks (104 tokens) then
            # slice the persistent d1s tile at pair boundaries
            d1s = cp.tile([128, TOK], f16)
            for s_ in range(N_SLICES):
                d1_ps = pp.tile([128, SLICE], f32,
                                tag="pc" if s_ % 2 else "pb", bufs=4,
                                name="d1ps")
                nc.tensor.matmul(
                    d1_ps, wd1T,
                    X[:, s_ * SLICE:(s_ + 1) * SLICE],
                    start=True, stop=True,
                )
                opa(d1s[:, s_ * SLICE:(s_ + 1) * SLICE], d1_ps, bd1_s,
                    zero_s)

            # y chunk = (W_d2 @ relu(d1))^T: stationary = d1s 104-token
            # chunk, moving = W_d2^T (6 cols) -> token-major y
            for m in range(TOK // (8 * PAIR_T)):
                if m % 8 == 0:
                    y_ps = pp.tile([PAIR_T, YCHUNKS * FEAT], f32,
                                   tag="pb", bufs=4)
                for p in range(8):
                    j = (m % 8) * 8 + p
                    c = m * 8 + p
                    nc.tensor.matmul(
                        y_ps[:, j * FEAT:(j + 1) * FEAT],
                        d1s[:, c * PAIR_T:(c + 1) * PAIR_T],
                        wd2T,
                        start=True, stop=True,
                    )
                if m % 8 == 7:
                    k = m // 8
                    y16 = wp.tile([PAIR_T, YCHUNKS * FEAT], f16, bufs=2)
                    nc.vector.tensor_copy(y16, y_ps)
                    dst = y_d[k * YCHUNKS * PAIR_T:
                              (k + 1) * YCHUNKS * PAIR_T, :]
                    nc.sync.dma_start(
                        dst.rearrange("(c t) f -> t c f", t=PAIR_T),
                        y16.rearrange("t (c f) -> t c f", f=FEAT),
                    )

    nc.finalize()
    return nc


def _host_prep(inputs):
    q = np.asarray(inputs["q_current"], np.float32).reshape(BATCH, N_NODES, FEAT)
    W_in = np.asarray(inputs["W_in"], np.float32)
    b_in = np.asarray(inputs["b_in"], np.float32)
    W_gnn = np.asarray(inputs["W_gnn"], np.float32)
    b_gnn = np.asarray(inputs["b_gnn"], np.float32)
    W_d1 = np.asarray(inputs["W_d1"], np.float32)
    b_d1 = np.asarray(inputs["b_d1"], np.float32)
    W_d2 = np.asarray(inputs["W_d2"], np.float32)
    edge = np.asarray(inputs["edge_index"]).astype(np.int64)

    # dense normalized adjacency (PyG GCNConv default w/ self-loops)
    loops = np.arange(N_NODES, dtype=np.int64)
    src = np.concatenate([edge[0], loops])
    dst = np.concatenate([edge[1], loops])
    deg = np.zeros(N_NODES, np.float32)
    np.add.at(deg, dst, 1.0)
    dinv = 1.0 / np.sqrt(np.maximum(deg, 1e-12))
    A = np.zeros((N_NODES, N_NODES), np.float32)
    np.add.at(A, (dst, src), dinv[src] * dinv[dst])

    # phase matrices: A_phi[t, 52*k + u] = A[u, node(phi + t)] for the
    # batch element k = (phi + t) // 52 the source token belongs to
    phis, uniq, pidx, plan, contrib = _phase_plan()
    n_ph = len(uniq)
    aph = np.zeros((n_ph, WIN, 4 * N_NODES), np.float32)
    for pi, phi in enumerate(uniq):
        for t in range(WIN):
            k = (phi + t) // N_NODES
            n = (phi + t) % N_NODES
            aph[pi, t, N_NODES * k:N_NODES * (k + 1)] = A[:, n]

    APH0 = 518
    blob = np.zeros((HID, APH0 + n_ph * 208), np.float32)
    blob[:, 0:384] = W_gnn.transpose(2, 0, 1).reshape(HID, N_LAYERS * HID)
    blob[:, 384:512] = W_d1.T
    blob[:, 512:518] = W_d2.T
    blob[:WIN, APH0:] = aph.transpose(1, 0, 2).reshape(WIN, n_ph * 208)

    biases = np.zeros((HID, 5), np.float32)
    biases[:, 0] = b_in
    biases[:, 1:4] = b_gnn.T
    biases[:, 4] = b_d1

    const = {
        "winT": np.ascontiguousarray(W_in.T).astype(np.float16),
        "blob16": blob.astype(np.float16),
        "biases": np.ascontiguousarray(biases),
        "vertag": np.zeros((1, _KVER), np.float32),
    }

    # per-core feature-major input [6, TOK], fp16
    q_flat = q.reshape(N_CORES, B_C * N_NODES, FEAT)
    qTs = [
        np.ascontiguousarray(q_flat[c].T).astype(np.float16)
        for c in range(N_CORES)
    ]
    return const, qTs


def kernel(**inputs):
    const, qTs = _host_prep(inputs)

    if "nc" not in _CACHE:
        _CACHE["nc"] = _build_nc()
    nc = _CACHE["nc"]

    from concourse.bass_utils import run_bass_kernel_spmd

    in_maps = [dict(const, qT=qTs[c]) for c in range(N_CORES)]
    res = run_bass_kernel_spmd(nc, in_maps, core_ids=list(range(N_CORES)))
    global _LAST_EXEC_NS
    _LAST_EXEC_NS = res.exec_time_ns
    if res.instructions_and_trace is not None:
        _LAST_TRACE.append(res.instructions_and_trace[1])

    b_d2 = np.asarray(inputs["b_d2"], np.float32)
    outs = []
    for c in range(N_CORES):
        yTm = res.results[c]["yTm"]  # [TOK, 6] fp16
        outs.append(np.asarray(yTm, np.float32) + b_d2)
    y = np.concatenate(outs, axis=0)  # [BATCH*52, 6]
    return np.ascontiguousarray(y).reshape(BATCH, N_FIL, N_SUB, FEAT)


# revision 33
# speedup vs baseline: 1.2510x; 1.0521x over previous
"""Batched GCN (microtubule dynamics model) on 8 Trainium2 NeuronCores.

Math: the reference's gather/scale/scatter message passing over a fixed
52-node graph is a dense linear operator on the node axis:
    agg[b] = A @ h[b],  A[i, j] = sum over edges (j->i, incl self-loops)
                                   of dinv[src] * dinv[dst]
and A commutes with the shared linear layer, so each GNN layer is
    x += relu((A @ x) @ W_l^T + b_l),  batched over B.

Device strategy (pure data parallel, 512 batch elems / core):
  - activations live in SBUF as [128 hid partitions, 26624 token cols]
    (token = b*52 + node), fp16 on-chip, fp32 PSUM for matmuls
  - per layer the token axis is processed in FULL 128-token windows
    (no per-batch-pair padding): fused transpose+W matmuls (stationary
    = 128-token X window, moving = W_l^T) fill one PSUM bank per 4
    windows -> ONE PSUM->SBUF copy per 512 tokens -> the node mix uses
    13 precomputed phase matrices A_phi[src 128, 4*52] (phi = window
    offset mod 52) as the moving operand, accumulating into per-bank
    agg PSUM tiles with start/stop groups -> relu+bias per full 512-col
    bank -> one accumulating SWDGE DMA per 4 banks does the residual
    (the SWDGE accumulate path corrupts beyond 4KB contiguous runs, so
    runs are kept at exactly 4KB)
  - PSUM-reading vector work cannot run on Pool (BIR: GPSIMD cannot
    access PSUM), so copies and relu+bias are greedily balanced over
    DVE and ACT by projected busy time; Pool only triggers SWDGE
  - decoder: d1 like the encoder; d2 exploits that matmul cost ~ moving
    free size: stationary = relu(d1) 104-token chunk, moving = W_d2^T
    (6 cols) -> token-major y in PSUM, copied once per 64 chunks and
    DMA'd to HBM as [TOK, 6] fp16; b_d2 is added during host unshard
"""

import numpy as np

N_FIL, N_SUB = 13, 4
N_NODES = N_FIL * N_SUB          # 52
FEAT = 6
HID = 128
N_LAYERS = 3
BATCH = 4096
N_CORES = 8
B_C = BATCH // N_CORES           # 512 batch elems per core
TOK = B_C * N_NODES              # 26624 token columns per core
PAIR_T = 2 * N_NODES             # 104 tokens per decoder d2 chunk
WIN = 128                        # tokens per transpose window
N_WIN = TOK // WIN               # 208 windows per layer
SLICE = 512                      # psum bank (fp32 cols)
N_SLICES = TOK // SLICE          # 52 banks of tokens
RB = 4                           # agg banks per residual accum DMA (4KB)
YCHUNKS = 64                     # d2 token-chunks per psum y tile

_CACHE = {}
_LAST_EXEC_NS = None
_LAST_TRACE = []
_DEBUG = False   # adds dbgX0/dbgX1 DRAM outputs (X after enc / after layer 1)
_KVER = 13       # bump on every semantic change: the execution service caches
                 # compiled NEFFs by program signature, and a changing input
                 # shape is the only reliable cache-buster


def _phase_plan():
    """Per-window node-mix plan against 512-token agg banks."""
    phis = [(WIN * w) % N_NODES for w in range(N_WIN)]
    uniq = sorted(set(phis))                      # 13 phases
    pidx = {p: i for i, p in enumerate(uniq)}
    plan = []                                     # w -> [(bank, olo, ohi, alo, ahi)]
    contrib = {}                                  # bank -> [w, ...]
    for w in range(N_WIN):
        e0 = (WIN * w) // N_NODES
        t_lo = N_NODES * e0
        t_hi = min(t_lo + 4 * N_NODES, TOK)
        ent = []
        for b in range(t_lo // SLICE, (t_hi - 1) // SLICE + 1):
            lo = max(t_lo, SLICE * b)
            hi = min(t_hi, SLICE * (b + 1))
            ent.append((b, lo - SLICE * b, hi - SLICE * b, lo - t_lo, hi - t_lo))
            contrib.setdefault(b, []).append(w)
        plan.append(ent)
    return phis, uniq, pidx, plan, contrib


def _build_nc():
    import concourse.bacc as bacc
    import concourse.mybir as mybir
    from concourse.tile import TileContext

    f32 = mybir.dt.float32
    f16 = mybir.dt.float16
    Alu = mybir.AluOpType
    Relu = mybir.ActivationFunctionType.Relu

    nc = bacc.Bacc(trn_type="TRN2")

    phis, uniq, pidx, plan, contrib = _phase_plan()
    n_ph = len(uniq)
    APH0 = 518                    # a2ph block starts here in the blob

    # blob16 cols: [0:384] wgT (3 layers x 128), [384:512] wd1T,
    # [512:518] wd2T, [518:518+13*208] A_phi phase matrices
    qT_d = nc.dram_tensor("qT", [FEAT, TOK], f16, kind="ExternalInput")
    winT_d = nc.dram_tensor("winT", [FEAT, HID], f16, kind="ExternalInput")
    blob_d = nc.dram_tensor("blob16", [HID, APH0 + n_ph * 208], f16,
                            kind="ExternalInput")
    # bias cols: 0 b_in, 1..3 b_gnn, 4 b_d1
    bias_d = nc.dram_tensor("biases", [HID, 5], f32, kind="ExternalInput")
    ver_d = nc.dram_tensor("vertag", [1, _KVER], f32, kind="ExternalInput")
    y_d = nc.dram_tensor("yTm", [TOK, FEAT], f16, kind="ExternalOutput")
    if _DEBUG:
        dbg0_d = nc.dram_tensor("dbgX0", [HID, TOK], f16, kind="ExternalOutput")
        dbg1_d = nc.dram_tensor("dbgX1", [HID, TOK], f16, kind="ExternalOutput")

    # greedy DVE/ACT balance: assign each PSUM-exit op to the engine with
    # the lower projected busy time (cost-model rates incl. fixed overheads)
    busy = {"A": 0.0, "D": 0.0}

    def next_eng(cols):
        ca = cols * 0.8333 + 215.0
        cd = cols * 1.0417 + 140.0
        if busy["A"] + ca / 2 <= busy["D"] + cd / 2:
            busy["A"] += ca
            return "A"
        busy["D"] += cd
        return "D"

    def opa(out, psum, bias_ap, zero):
        # out = relu(psum + bias) on DVE or ACT
        if next_eng(out.shape[-1]) == "A":
            nc.scalar.activation(out, psum, Relu, bias=bias_ap)
        else:
            nc.vector.scalar_tensor_tensor(
                out, psum, bias_ap, zero[:, :out.shape[-1]],
                op0=Alu.add, op1=Alu.max,
            )

    def copy(out, psum):
        if next_eng(out.shape[-1]) == "A":
            nc.scalar.copy(out, psum)
        else:
            nc.vector.tensor_copy(out, psum)

    with TileContext(nc) as tc:
        with (
            tc.sbuf_pool(name="const", bufs=1) as cp,
            tc.sbuf_pool(name="work", bufs=4) as wp,
            tc.psum_pool(name="ps", bufs=2) as pp,
        ):
            blob = cp.tile_from(blob_d[:, :])
            winT = cp.tile_from(winT_d[:, :])
            biases = cp.tile_from(bias_d[:, :])
            vtag = cp.tile([1, _KVER], f32)
            nc.sync.dma_start(vtag, ver_d[:, :])
            zero_s = cp.tile([128, 2 * SLICE], f16)
            nc.vector.memset(zero_s, 0.0)
            # warmup: trigger the one-time ACT function-table load while the
            # qT input DMA is still in flight
            warm = cp.tile([1, _KVER], f32)
            nc.scalar.activation(warm, vtag, Relu)

            wd1T = blob[:, 384:512]
            wd2T = blob[:, 512:518]
            bin_s = biases[:, 0:1]
            bd1_s = biases[:, 4:5]

            qT = cp.tile([FEAT, TOK], f16)
            nc.sync.dma_start(qT, qT_d[:, :])

            X = cp.tile([128, TOK], f16)

            # -------- encoder: X = relu(W_in @ q^T + b_in) ----------------
            for t in range(N_SLICES // 2):
                if t % 2:
                    pa = pp.tile([128, SLICE], f32, tag="pc", bufs=4,
                                 name="encps1")
                    pb_ = pp.tile([128, SLICE], f32, tag="pc", bufs=4,
                                  name="encps2")
                    parts = [pa, pb_]
                else:
                    enc_ps = pp.tile([128, 2 * SLICE], f32, tag="pb", bufs=2,
                                     name="encps")
                    parts = [enc_ps[:, :SLICE], enc_ps[:, SLICE:]]
                for q in range(2):
                    s_ = 2 * t + q
                    nc.tensor.matmul(
                        parts[q], winT,
                        qT[:, s_ * SLICE:(s_ + 1) * SLICE],
                        start=True, stop=True,
                    )
                if t % 2:
                    for q in range(2):
                        s_ = 2 * t + q
                        opa(X[:, s_ * SLICE:(s_ + 1) * SLICE], parts[q],
                            bin_s, zero_s)
                else:
                    opa(X[:, t * 2 * SLICE:(t + 1) * 2 * SLICE], enc_ps,
                        bin_s, zero_s)

            if _DEBUG:
                nc.sync.dma_start(dbg0_d[:, :], X)

            # -------- GNN layers: x += relu(A (x W_l^T) + b_l) -----------
            # software-pipelined: the transpose+W matmuls for tile T are
            # emitted before the node mix of tile T-1 so the in-order PE
            # queue never stalls on the PSUM->SBUF copy
            for l in range(N_LAYERS):
                wgT_l = blob[:, l * 128:(l + 1) * 128]
                bg_l = biases[:, 1 + l:2 + l]
                ht_tiles = {}
                agg_open = {}
                done_banks = []
                r8 = None
                for T in range(N_SLICES // 2 + 1):       # 26 2-bank tiles
                    if T < N_SLICES // 2:
                        ht_ps = pp.tile([128, 2 * SLICE], f32, tag="pb",
                                        bufs=2, name="htps")
                        for i in range(8):
                            w = 8 * T + i
                            nc.tensor.matmul(
                                ht_ps[:, i * WIN:(i + 1) * WIN],
                                X[:, w * WIN:(w + 1) * WIN],
                                wgT_l,
                                start=True, stop=True,
                            )
                        ht_tiles[T] = ht_ps
                    if T == 0:
                        continue
                    TT = T - 1
                    hts = wp.tile([128, 2 * SLICE], f16, bufs=6)
                    copy(hts, ht_tiles.pop(TT))

                    # node mix: moving = A_phi slices, accumulating into
                    # 1-bank agg tiles with start/stop groups; enc/d1 share
                    # the pc tag so agg tiles use their own tag "pa"? no --
                    # pc is 2-bank bufs=2 for enc/d1, aggs use tag "pa"
                    for i in range(8):
                        w = 8 * TT + i
                        aph = blob[:, APH0 + pidx[phis[w]] * 208:
                                      APH0 + (pidx[phis[w]] + 1) * 208]
                        for (b, olo, ohi, alo, ahi) in plan[w]:
                            if b not in agg_open:
                                agg_open[b] = pp.tile(
                                    [128, SLICE], f32, tag="pc", bufs=4,
                                    name="aggps",
                                )
                            nc.tensor.matmul(
                                agg_open[b][:, olo:ohi],
                                hts[:, i * WIN:(i + 1) * WIN],
                                aph[:, alo:ahi],
                                start=(w == contrib[b][0]),
                                stop=(w == contrib[b][-1]),
                            )
                            if w == contrib[b][-1]:
                                done_banks.append(b)

                    # banks complete by the PREVIOUS tile are relu+bias'd
                    # now (emission delay keeps the engine queues from
                    # reaching an opA before its matmuls), then one
                    # accumulating SWDGE DMA per RB banks (4KB runs)
                    flush = (T == N_SLICES // 2)
                    while done_banks and (flush or len(done_banks) > 2):
                        b = done_banks.pop(0)
                        if b % RB == 0:
                            r8 = wp.tile([128, RB * SLICE], f16, bufs=3,
                                         name="r8")
                        opa(r8[:, (b % RB) * SLICE:(b % RB + 1) * SLICE],
                            agg_open.pop(b), bg_l, zero_s)
                        if b % RB == RB - 1:
                            nc.gpsimd.dma_start(
                                X[:, (b - RB + 1) * SLICE:(b + 1) * SLICE],
                                r8, accum_op=Alu.add,
                            )
                    assert len(agg_open) <= 4, (T, sorted(agg_open))
                if _DEBUG and l == 0:
                    nc.sync.dma_start(dbg1_d[:, :], X)

            # -------- decoder --------------------------------------------
            # d1 slices are batch-agnostic; d2 chunks (104 tokens) then
            # slice the persistent d1s tile at pair boundaries. d2 work is
            # emitted interleaved with d1 (per 8-chunk m-group, as soon as
            # its d1s tokens are written) so the tiny d2 matmuls hide under
            # d1's vector work instead of forming a serial tail
            d1s = cp.tile([128, TOK], f16)
            y16 = cp.tile([PAIR_T, (TOK // PAIR_T) * FEAT], f16)
            g_emitted = 0

            def emit_d2_upto(covered):
                nonlocal g_emitted
                while g_emitted < TOK // (8 * PAIR_T) and \
                        (g_emitted + 1) * 8 * PAIR_T <= covered:
                    g = g_emitted
                    y_ps = pp.tile([PAIR_T, 8 * FEAT], f32, tag="pc",
                                   bufs=4, name="yps")
                    for p in range(8):
                        c = g * 8 + p
                        nc.tensor.matmul(
                            y_ps[:, p * FEAT:(p + 1) * FEAT],
                            d1s[:, c * PAIR_T:(c + 1) * PAIR_T],
                            wd2T,
                            start=True, stop=True,
                        )
                    copy(y16[:, g * 8 * FEAT:(g + 1) * 8 * FEAT], y_ps)
                    if (g + 1) % 8 == 0:
                        k = g // 8
                        dst = y_d[k * YCHUNKS * PAIR_T:
                                  (k + 1) * YCHUNKS * PAIR_T, :]
                        nc.sync.dma_start(
                            dst.rearrange("(c t) f -> t c f", t=PAIR_T),
                            y16[:, k * YCHUNKS * FEAT:
                                   (k + 1) * YCHUNKS * FEAT].rearrange(
                                "t (c f) -> t c f", f=FEAT),
                        )
                    g_emitted += 1

            for t in range(N_SLICES // 2):
                if t % 2:
                    pa = pp.tile([128, SLICE], f32, tag="pc", bufs=4,
                                 name="d1ps1")
                    pb_ = pp.tile([128, SLICE], f32, tag="pc", bufs=4,
                                  name="d1ps2")
                    parts = [pa, pb_]
                else:
                    d1_ps = pp.tile([128, 2 * SLICE], f32, tag="pb", bufs=2,
                                    name="d1ps")
                    parts = [d1_ps[:, :SLICE], d1_ps[:, SLICE:]]
                for q in range(2):
                    s_ = 2 * t + q
                    nc.tensor.matmul(
                        parts[q], wd1T,
                        X[:, s_ * SLICE:(s_ + 1) * SLICE],
                        start=True, stop=True,
                    )
                if t % 2:
                    for q in range(2):
                        s_ = 2 * t + q
                        opa(d1s[:, s_ * SLICE:(s_ + 1) * SLICE], parts[q],
                            bd1_s, zero_s)
                else:
                    opa(d1s[:, t * 2 * SLICE:(t + 1) * 2 * SLICE], d1_ps,
                        bd1_s, zero_s)
                emit_d2_upto(2 * SLICE * t)   # one tile of emission delay

            emit_d2_upto(TOK)

    nc.finalize()
    return nc


def _host_prep(inputs):
    q = np.asarray(inputs["q_current"], np.float32).reshape(BATCH, N_NODES, FEAT)
    W_in = np.asarray(inputs["W_in"], np.float32)
    b_in = np.asarray(inputs["b_in"], np.float32)
    W_gnn = np.asarray(inputs["W_gnn"], np.float32)
    b_gnn = np.asarray(inputs["b_gnn"], np.float32)
    W_d1 = np.asarray(inputs["W_d1"], np.float32)
    b_d1 = np.asarray(inputs["b_d1"], np.float32)
    W_d2 = np.asarray(inputs["W_d2"], np.float32)
    edge = np.asarray(inputs["edge_index"]).astype(np.int64)

    # dense normalized adjacency (PyG GCNConv default w/ self-loops)
    loops = np.arange(N_NODES, dtype=np.int64)
    src = np.concatenate([edge[0], loops])
    dst = np.concatenate([edge[1], loops])
    deg = np.zeros(N_NODES, np.float32)
    np.add.at(deg, dst, 1.0)
    dinv = 1.0 / np.sqrt(np.maximum(deg, 1e-12))
    A = np.zeros((N_NODES, N_NODES), np.float32)
    np.add.at(A, (dst, src), dinv[src] * dinv[dst])

    # phase matrices: A_phi[t, 52*k + u] = A[u, node(phi + t)] for the
    # batch element k = (phi + t) // 52 the source token belongs to
    phis, uniq, pidx, plan, contrib = _phase_plan()
    n_ph = len(uniq)
    aph = np.zeros((n_ph, WIN, 4 * N_NODES), np.float32)
    for pi, phi in enumerate(uniq):
        for t in range(WIN):
            k = (phi + t) // N_NODES
            n = (phi + t) % N_NODES
            aph[pi, t, N_NODES * k:N_NODES * (k + 1)] = A[:, n]

    APH0 = 518
    blob = np.zeros((HID, APH0 + n_ph * 208), np.float32)
    blob[:, 0:384] = W_gnn.transpose(2, 0, 1).reshape(HID, N_LAYERS * HID)
    blob[:, 384:512] = W_d1.T
    blob[:, 512:518] = W_d2.T
    blob[:WIN, APH0:] = aph.transpose(1, 0, 2).reshape(WIN, n_ph * 208)

    biases = np.zeros((HID, 5), np.float32)
    biases[:, 0] = b_in
    biases[:, 1:4] = b_gnn.T
    biases[:, 4] = b_d1

    const = {
        "winT": np.ascontiguousarray(W_in.T).astype(np.float16),
        "blob16": blob.astype(np.float16),
        "biases": np.ascontiguousarray(biases),
        "vertag": np.zeros((1, _KVER), np.float32),
    }

    # per-core feature-major input [6, TOK], fp16
    q_flat = q.reshape(N_CORES, B_C * N_NODES, FEAT)
    qTs = [
        np.ascontiguousarray(q_flat[c].T).astype(np.float16)
        for c in range(N_CORES)
    ]
    return const, qTs


def kernel(**inputs):
    const, qTs = _host_prep(inputs)

    if "nc" not in _CACHE:
        _CACHE["nc"] = _build_nc()
    nc = _CACHE["nc"]

    from concourse.bass_utils import run_bass_kernel_spmd

    in_maps = [dict(const, qT=qTs[c]) for c in range(N_CORES)]
    res = run_bass_kernel_spmd(nc, in_maps, core_ids=list(range(N_CORES)))
    global _LAST_EXEC_NS
    _LAST_EXEC_NS = res.exec_time_ns
    if res.instructions_and_trace is not None:
        _LAST_TRACE.append(res.instructions_and_trace[1])

    b_d2 = np.asarray(inputs["b_d2"], np.float32)
    outs = []
    for c in range(N_CORES):
        yTm = res.results[c]["yTm"]  # [TOK, 6] fp16
        outs.append(np.asarray(yTm, np.float32) + b_d2)
    y = np.concatenate(outs, axis=0)  # [BATCH*52, 6]
    return np.ascontiguousarray(y).reshape(BATCH, N_FIL, N_SUB, FEAT)


# revision 34
# speedup vs baseline: 1.2542x; 1.0025x over previous
"""Batched GCN (microtubule dynamics model) on 8 Trainium2 NeuronCores.

Math: the reference's gather/scale/scatter message passing over a fixed
52-node graph is a dense linear operator on the node axis:
    agg[b] = A @ h[b],  A[i, j] = sum over edges (j->i, incl self-loops)
                                   of dinv[src] * dinv[dst]
and A commutes with the shared linear layer, so each GNN layer is
    x += relu((A @ x) @ W_l^T + b_l),  batched over B.

Device strategy (pure data parallel, 512 batch elems / core):
  - activations live in SBUF as [128 hid partitions, 26624 token cols]
    (token = b*52 + node), fp16 on-chip, fp32 PSUM for matmuls
  - per layer the token axis is processed in FULL 128-token windows
    (no per-batch-pair padding): fused transpose+W matmuls (stationary
    = 128-token X window, moving = W_l^T) fill one PSUM bank per 4
    windows -> ONE PSUM->SBUF copy per 512 tokens -> the node mix uses
    13 precomputed phase matrices A_phi[src 128, 4*52] (phi = window
    offset mod 52) as the moving operand, accumulating into per-bank
    agg PSUM tiles with start/stop groups -> relu+bias per full 512-col
    bank -> one accumulating SWDGE DMA per 4 banks does the residual
    (the SWDGE accumulate path corrupts beyond 4KB contiguous runs, so
    runs are kept at exactly 4KB)
  - PSUM-reading vector work cannot run on Pool (BIR: GPSIMD cannot
    access PSUM), so copies and relu+bias are greedily balanced over
    DVE and ACT by projected busy time; Pool only triggers SWDGE
  - decoder: d1 like the encoder; d2 exploits that matmul cost ~ moving
    free size: stationary = relu(d1) 104-token chunk, moving = W_d2^T
    (6 cols) -> token-major y in PSUM, copied once per 64 chunks and
    DMA'd to HBM as [TOK, 6] fp16; b_d2 is added during host unshard
"""

import numpy as np

N_FIL, N_SUB = 13, 4
N_NODES = N_FIL * N_SUB          # 52
FEAT = 6
HID = 128
N_LAYERS = 3
BATCH = 4096
N_CORES = 8
B_C = BATCH // N_CORES           # 512 batch elems per core
TOK = B_C * N_NODES              # 26624 token columns per core
PAIR_T = 2 * N_NODES             # 104 tokens per decoder d2 chunk
WIN = 128                        # tokens per transpose window
N_WIN = TOK // WIN               # 208 windows per layer
SLICE = 512                      # psum bank (fp32 cols)
N_SLICES = TOK // SLICE          # 52 banks of tokens
RB = 4                           # agg banks per residual accum DMA (4KB)
YCHUNKS = 64                     # d2 token-chunks per psum y tile

_CACHE = {}
_LAST_EXEC_NS = None
_LAST_TRACE = []
_DEBUG = False   # adds dbgX0/dbgX1 DRAM outputs (X after enc / after layer 1)
_KVER = 14       # bump on every semantic change: the execution service caches
                 # compiled NEFFs by program signature, and a changing input
                 # shape is the only reliable cache-buster


def _phase_plan():
    """Per-window node-mix plan against 512-token agg banks."""
    phis = [(WIN * w) % N_NODES for w in range(N_WIN)]
    uniq = sorted(set(phis))                      # 13 phases
    pidx = {p: i for i, p in enumerate(uniq)}
    plan = []                                     # w -> [(bank, olo, ohi, alo, ahi)]
    contrib = {}                                  # bank -> [w, ...]
    for w in range(N_WIN):
        e0 = (WIN * w) // N_NODES
        t_lo = N_NODES * e0
        t_hi = min(t_lo + 4 * N_NODES, TOK)
        ent = []
        for b in range(t_lo // SLICE, (t_hi - 1) // SLICE + 1):
            lo = max(t_lo, SLICE * b)
            hi = min(t_hi, SLICE * (b + 1))
            ent.append((b, lo - SLICE * b, hi - SLICE * b, lo - t_lo, hi - t_lo))
            contrib.setdefault(b, []).append(w)
        plan.append(ent)
    return phis, uniq, pidx, plan, contrib


def _build_nc():
    import concourse.bacc as bacc
    import concourse.mybir as mybir
    from concourse.tile import TileContext

    f32 = mybir.dt.float32
    f16 = mybir.dt.float16
    Alu = mybir.AluOpType
    Relu = mybir.ActivationFunctionType.Relu

    nc = bacc.Bacc(trn_type="TRN2")

    phis, uniq, pidx, plan, contrib = _phase_plan()
    n_ph = len(uniq)
    APH0 = 518                    # a2ph block starts here in the blob

    # blob16 cols: [0:384] wgT (3 layers x 128), [384:512] wd1T,
    # [512:518] wd2T, [518:518+13*208] A_phi phase matrices
    qT_d = nc.dram_tensor("qT", [FEAT, TOK], f16, kind="ExternalInput")
    winT_d = nc.dram_tensor("winT", [FEAT, HID], f16, kind="ExternalInput")
    blob_d = nc.dram_tensor("blob16", [HID, APH0 + n_ph * 208], f16,
                            kind="ExternalInput")
    # bias cols: 0 b_in, 1..3 b_gnn, 4 b_d1
    bias_d = nc.dram_tensor("biases", [HID, 5], f32, kind="ExternalInput")
    ver_d = nc.dram_tensor("vertag", [1, _KVER], f32, kind="ExternalInput")
    y_d = nc.dram_tensor("yTm", [TOK, FEAT], f16, kind="ExternalOutput")
    if _DEBUG:
        dbg0_d = nc.dram_tensor("dbgX0", [HID, TOK], f16, kind="ExternalOutput")
        dbg1_d = nc.dram_tensor("dbgX1", [HID, TOK], f16, kind="ExternalOutput")

    # greedy DVE/ACT balance: assign each PSUM-exit op to the engine with
    # the lower projected busy time (cost-model rates incl. fixed overheads)
    busy = {"A": 0.0, "D": 0.0}

    def next_eng(cols):
        ca = cols * 0.8333 + 215.0
        cd = cols * 1.0417 + 140.0
        if busy["A"] + ca / 2 <= busy["D"] + cd / 2:
            busy["A"] += ca
            return "A"
        busy["D"] += cd
        return "D"

    def opa(out, psum, bias_ap, zero):
        # out = relu(psum + bias) on DVE or ACT
        if next_eng(out.shape[-1]) == "A":
            nc.scalar.activation(out, psum, Relu, bias=bias_ap)
        else:
            nc.vector.scalar_tensor_tensor(
                out, psum, bias_ap, zero[:, :out.shape[-1]],
                op0=Alu.add, op1=Alu.max,
            )

    def copy(out, psum):
        if next_eng(out.shape[-1]) == "A":
            nc.scalar.copy(out, psum)
        else:
            nc.vector.tensor_copy(out, psum)

    with TileContext(nc) as tc:
        with (
            tc.sbuf_pool(name="const", bufs=1) as cp,
            tc.sbuf_pool(name="work", bufs=4) as wp,
            tc.psum_pool(name="ps", bufs=2) as pp,
        ):
            blob = cp.tile_from(blob_d[:, :])
            winT = cp.tile_from(winT_d[:, :])
            biases = cp.tile_from(bias_d[:, :])
            vtag = cp.tile([1, _KVER], f32)
            nc.sync.dma_start(vtag, ver_d[:, :])
            zero_s = cp.tile([128, 2 * SLICE], f16)
            nc.vector.memset(zero_s, 0.0)
            # warmup: trigger the one-time ACT function-table load while the
            # qT input DMA is still in flight
            warm = cp.tile([1, _KVER], f32)
            nc.scalar.activation(warm, vtag, Relu)

            wd1T = blob[:, 384:512]
            wd2T = blob[:, 512:518]
            bin_s = biases[:, 0:1]
            bd1_s = biases[:, 4:5]

            qT = cp.tile([FEAT, TOK], f16)
            for c in range(4):
                nc.sync.dma_start(
                    qT[:, c * (TOK // 4):(c + 1) * (TOK // 4)],
                    qT_d[:, c * (TOK // 4):(c + 1) * (TOK // 4)],
                )

            X = cp.tile([128, TOK], f16)

            # -------- encoder: X = relu(W_in @ q^T + b_in) ----------------
            for t in range(N_SLICES // 2):
                if t % 2:
                    pa = pp.tile([128, SLICE], f32, tag="pc", bufs=4,
                                 name="encps1")
                    pb_ = pp.tile([128, SLICE], f32, tag="pc", bufs=4,
                                  name="encps2")
                    parts = [pa, pb_]
                else:
                    enc_ps = pp.tile([128, 2 * SLICE], f32, tag="pb", bufs=2,
                                     name="encps")
                    parts = [enc_ps[:, :SLICE], enc_ps[:, SLICE:]]
                for q in range(2):
                    s_ = 2 * t + q
                    nc.tensor.matmul(
                        parts[q], winT,
                        qT[:, s_ * SLICE:(s_ + 1) * SLICE],
                        start=True, stop=True,
                    )
                if t % 2:
                    for q in range(2):
                        s_ = 2 * t + q
                        opa(X[:, s_ * SLICE:(s_ + 1) * SLICE], parts[q],
                            bin_s, zero_s)
                else:
                    opa(X[:, t * 2 * SLICE:(t + 1) * 2 * SLICE], enc_ps,
                        bin_s, zero_s)

            if _DEBUG:
                nc.sync.dma_start(dbg0_d[:, :], X)

            # -------- GNN layers: x += relu(A (x W_l^T) + b_l) -----------
            # software-pipelined: the transpose+W matmuls for tile T are
            # emitted before the node mix of tile T-1 so the in-order PE
            # queue never stalls on the PSUM->SBUF copy
            for l in range(N_LAYERS):
                wgT_l = blob[:, l * 128:(l + 1) * 128]
                bg_l = biases[:, 1 + l:2 + l]
                ht_tiles = {}
                agg_open = {}
                done_banks = []
                r8 = None
                for T in range(N_SLICES // 2 + 1):       # 26 2-bank tiles
                    if T < N_SLICES // 2:
                        ht_ps = pp.tile([128, 2 * SLICE], f32, tag="pb",
                                        bufs=2, name="htps")
                        for i in range(8):
                            w = 8 * T + i
                            nc.tensor.matmul(
                                ht_ps[:, i * WIN:(i + 1) * WIN],
                                X[:, w * WIN:(w + 1) * WIN],
                                wgT_l,
                                start=True, stop=True,
                            )
                        ht_tiles[T] = ht_ps
                    if T == 0:
                        continue
                    TT = T - 1
                    hts = wp.tile([128, 2 * SLICE], f16, bufs=6)
                    copy(hts, ht_tiles.pop(TT))

                    # node mix: moving = A_phi slices, accumulating into
                    # 1-bank agg tiles with start/stop groups; enc/d1 share
                    # the pc tag so agg tiles use their own tag "pa"? no --
                    # pc is 2-bank bufs=2 for enc/d1, aggs use tag "pa"
                    for i in range(8):
                        w = 8 * TT + i
                        aph = blob[:, APH0 + pidx[phis[w]] * 208:
                                      APH0 + (pidx[phis[w]] + 1) * 208]
                        for (b, olo, ohi, alo, ahi) in plan[w]:
                            if b not in agg_open:
                                agg_open[b] = pp.tile(
                                    [128, SLICE], f32, tag="pc", bufs=4,
                                    name="aggps",
                                )
                            nc.tensor.matmul(
                                agg_open[b][:, olo:ohi],
                                hts[:, i * WIN:(i + 1) * WIN],
                                aph[:, alo:ahi],
                                start=(w == contrib[b][0]),
                                stop=(w == contrib[b][-1]),
                            )
                            if w == contrib[b][-1]:
                                done_banks.append(b)

                    # banks complete by the PREVIOUS tile are relu+bias'd
                    # now (emission delay keeps the engine queues from
                    # reaching an opA before its matmuls), then one
                    # accumulating SWDGE DMA per RB banks (4KB runs)
                    flush = (T == N_SLICES // 2)
                    while done_banks and (flush or len(done_banks) > 2):
                        b = done_banks.pop(0)
                        if b % RB == 0:
                            r8 = wp.tile([128, RB * SLICE], f16, bufs=3,
                                         name="r8")
                        opa(r8[:, (b % RB) * SLICE:(b % RB + 1) * SLICE],
                            agg_open.pop(b), bg_l, zero_s)
                        if b % RB == RB - 1:
                            nc.gpsimd.dma_start(
                                X[:, (b - RB + 1) * SLICE:(b + 1) * SLICE],
                                r8, accum_op=Alu.add,
                            )
                    assert len(agg_open) <= 4, (T, sorted(agg_open))
                if _DEBUG and l == 0:
                    nc.sync.dma_start(dbg1_d[:, :], X)

            # -------- decoder --------------------------------------------
            # d1 slices are batch-agnostic; d2 chunks (104 tokens) then
            # slice the persistent d1s tile at pair boundaries. d2 work is
            # emitted interleaved with d1 (per 8-chunk m-group, as soon as
            # its d1s tokens are written) so the tiny d2 matmuls hide under
            # d1's vector work instead of forming a serial tail
            d1s = cp.tile([128, TOK], f16)
            y16 = cp.tile([PAIR_T, (TOK // PAIR_T) * FEAT], f16)
            g_emitted = 0

            def emit_d2_upto(covered):
                nonlocal g_emitted
                while g_emitted < TOK // (8 * PAIR_T) and \
                        (g_emitted + 1) * 8 * PAIR_T <= covered:
                    g = g_emitted
                    y_ps = pp.tile([PAIR_T, 8 * FEAT], f32, tag="pc",
                                   bufs=4, name="yps")
                    for p in range(8):
                        c = g * 8 + p
                        nc.tensor.matmul(
                            y_ps[:, p * FEAT:(p + 1) * FEAT],
                            d1s[:, c * PAIR_T:(c + 1) * PAIR_T],
                            wd2T,
                            start=True, stop=True,
                        )
                    copy(y16[:, g * 8 * FEAT:(g + 1) * 8 * FEAT], y_ps)
                    if (g + 1) % 8 == 0:
                        k = g // 8
                        dst = y_d[k * YCHUNKS * PAIR_T:
                                  (k + 1) * YCHUNKS * PAIR_T, :]
                        nc.sync.dma_start(
                            dst.rearrange("(c t) f -> t c f", t=PAIR_T),
                            y16[:, k * YCHUNKS * FEAT:
                                   (k + 1) * YCHUNKS * FEAT].rearrange(
                                "t (c f) -> t c f", f=FEAT),
                        )
                    g_emitted += 1

            for t in range(N_SLICES // 2):
                if t % 2:
                    pa = pp.tile([128, SLICE], f32, tag="pc", bufs=4,
                                 name="d1ps1")
                    pb_ = pp.tile([128, SLICE], f32, tag="pc", bufs=4,
                                  name="d1ps2")
                    parts = [pa, pb_]
                else:
                    d1_ps = pp.tile([128, 2 * SLICE], f32, tag="pb", bufs=2,
                                    name="d1ps")
                    parts = [d1_ps[:, :SLICE], d1_ps[:, SLICE:]]
                for q in range(2):
                    s_ = 2 * t + q
                    nc.tensor.matmul(
                        parts[q], wd1T,
                        X[:, s_ * SLICE:(s_ + 1) * SLICE],
                        start=True, stop=True,
                    )
                if t % 2:
                    for q in range(2):
                        s_ = 2 * t + q
                        opa(d1s[:, s_ * SLICE:(s_ + 1) * SLICE], parts[q],
                            bd1_s, zero_s)
                else:
                    opa(d1s[:, t * 2 * SLICE:(t + 1) * 2 * SLICE], d1_ps,
                        bd1_s, zero_s)
                emit_d2_upto(2 * SLICE * t)   # one tile of emission delay

            emit_d2_upto(TOK)

    nc.finalize()
    return nc


def _host_prep(inputs):
    q = np.asarray(inputs["q_current"], np.float32).reshape(BATCH, N_NODES, FEAT)
    W_in = np.asarray(inputs["W_in"], np.float32)
    b_in = np.asarray(inputs["b_in"], np.float32)
    W_gnn = np.asarray(inputs["W_gnn"], np.float32)
    b_gnn = np.asarray(inputs["b_gnn"], np.float32)
    W_d1 = np.asarray(inputs["W_d1"], np.float32)
    b_d1 = np.asarray(inputs["b_d1"], np.float32)
    W_d2 = np.asarray(inputs["W_d2"], np.float32)
    edge = np.asarray(inputs["edge_index"]).astype(np.int64)

    # dense normalized adjacency (PyG GCNConv default w/ self-loops)
    loops = np.arange(N_NODES, dtype=np.int64)
    src = np.concatenate([edge[0], loops])
    dst = np.concatenate([edge[1], loops])
    deg = np.zeros(N_NODES, np.float32)
    np.add.at(deg, dst, 1.0)
    dinv = 1.0 / np.sqrt(np.maximum(deg, 1e-12))
    A = np.zeros((N_NODES, N_NODES), np.float32)
    np.add.at(A, (dst, src), dinv[src] * dinv[dst])

    # phase matrices: A_phi[t, 52*k + u] = A[u, node(phi + t)] for the
    # batch element k = (phi + t) // 52 the source token belongs to
    phis, uniq, pidx, plan, contrib = _phase_plan()
    n_ph = len(uniq)
    aph = np.zeros((n_ph, WIN, 4 * N_NODES), np.float32)
    for pi, phi in enumerate(uniq):
        for t in range(WIN):
            k = (phi + t) // N_NODES
            n = (phi + t) % N_NODES
            aph[pi, t, N_NODES * k:N_NODES * (k + 1)] = A[:, n]

    APH0 = 518
    blob = np.zeros((HID, APH0 + n_ph * 208), np.float32)
    blob[:, 0:384] = W_gnn.transpose(2, 0, 1).reshape(HID, N_LAYERS * HID)
    blob[:, 384:512] = W_d1.T
    blob[:, 512:518] = W_d2.T
    blob[:WIN, APH0:] = aph.transpose(1, 0, 2).reshape(WIN, n_ph * 208)

    biases = np.zeros((HID, 5), np.float32)
    biases[:, 0] = b_in
    biases[:, 1:4] = b_gnn.T
    biases[:, 4] = b_d1

    const = {
        "winT": np.ascontiguousarray(W_in.T).astype(np.float16),
        "blob16": blob.astype(np.float16),
        "biases": np.ascontiguousarray(biases),
        "vertag": np.zeros((1, _KVER), np.float32),
    }

    # per-core feature-major input [6, TOK], fp16
    q_flat = q.reshape(N_CORES, B_C * N_NODES, FEAT)
    qTs = [
        np.ascontiguousarray(q_flat[c].T).astype(np.float16)
        for c in range(N_CORES)
    ]
    return const, qTs


def kernel(**inputs):
    const, qTs = _host_prep(inputs)

    if "nc" not in _CACHE:
        _CACHE["nc"] = _build_nc()
    nc = _CACHE["nc"]

    from concourse.bass_utils import run_bass_kernel_spmd

    in_maps = [dict(const, qT=qTs[c]) for c in range(N_CORES)]
    res = run_bass_kernel_spmd(nc, in_maps, core_ids=list(range(N_CORES)))
    global _LAST_EXEC_NS
    _LAST_EXEC_NS = res.exec_time_ns
    if res.instructions_and_trace is not None:
        _LAST_TRACE.append(res.instructions_and_trace[1])

    b_d2 = np.asarray(inputs["b_d2"], np.float32)
    outs = []
    for c in range(N_CORES):
        yTm = res.results[c]["yTm"]  # [TOK, 6] fp16
        outs.append(np.asarray(yTm, np.float32) + b_d2)
    y = np.concatenate(outs, axis=0)  # [BATCH*52, 6]
    return np.ascontiguousarray(y).reshape(BATCH, N_FIL, N_SUB, FEAT)


# revision 37
# speedup vs baseline: 1.2655x; 1.0090x over previous
"""Batched GCN (microtubule dynamics model) on 8 Trainium2 NeuronCores.

Math: the reference's gather/scale/scatter message passing over a fixed
52-node graph is a dense linear operator on the node axis:
    agg[b] = A @ h[b],  A[i, j] = sum over edges (j->i, incl self-loops)
                                   of dinv[src] * dinv[dst]
and A commutes with the shared linear layer, so each GNN layer is
    x += relu((A @ x) @ W_l^T + b_l),  batched over B.

Device strategy (pure data parallel, 512 batch elems / core):
  - activations live in SBUF as [128 hid partitions, 26624 token cols]
    (token = b*52 + node), fp16 on-chip, fp32 PSUM for matmuls
  - per layer the token axis is processed in FULL 128-token windows
    (no per-batch-pair padding): fused transpose+W matmuls (stationary
    = 128-token X window, moving = W_l^T) fill one PSUM bank per 4
    windows -> ONE PSUM->SBUF copy per 512 tokens -> the node mix uses
    13 precomputed phase matrices A_phi[src 128, 4*52] (phi = window
    offset mod 52) as the moving operand, accumulating into per-bank
    agg PSUM tiles with start/stop groups -> relu+bias per full 512-col
    bank -> one accumulating SWDGE DMA per 4 banks does the residual
    (the SWDGE accumulate path corrupts beyond 4KB contiguous runs, so
    runs are kept at exactly 4KB)
  - PSUM-reading vector work cannot run on Pool (BIR: GPSIMD cannot
    access PSUM), so copies and relu+bias are greedily balanced over
    DVE and ACT by projected busy time; Pool only triggers SWDGE
  - decoder: d1 like the encoder; d2 exploits that matmul cost ~ moving
    free size: stationary = relu(d1) 104-token chunk, moving = W_d2^T
    (6 cols) -> token-major y in PSUM, copied once per 64 chunks and
    DMA'd to HBM as [TOK, 6] fp16; b_d2 is added during host unshard
"""

import numpy as np

N_FIL, N_SUB = 13, 4
N_NODES = N_FIL * N_SUB          # 52
FEAT = 6
HID = 128
N_LAYERS = 3
BATCH = 4096
N_CORES = 8
B_C = BATCH // N_CORES           # 512 batch elems per core
TOK = B_C * N_NODES              # 26624 token columns per core
PAIR_T = 2 * N_NODES             # 104 tokens per decoder d2 chunk
WIN = 128                        # tokens per transpose window
N_WIN = TOK // WIN               # 208 windows per layer
SLICE = 512                      # psum bank (fp32 cols)
N_SLICES = TOK // SLICE          # 52 banks of tokens
RB = 4                           # agg banks per residual accum DMA (4KB)
YCHUNKS = 64                     # d2 token-chunks per psum y tile

_CACHE = {}
_LAST_EXEC_NS = None
_LAST_TRACE = []
_DEBUG = False   # adds dbgX0/dbgX1 DRAM outputs (X after enc / after layer 1)
_KVER = 15       # bump on every semantic change: the execution service caches
                 # compiled NEFFs by program signature, and a changing input
                 # shape is the only reliable cache-buster


def _phase_plan():
    """Per-window node-mix plan against 512-token agg banks."""
    phis = [(WIN * w) % N_NODES for w in range(N_WIN)]
    uniq = sorted(set(phis))                      # 13 phases
    pidx = {p: i for i, p in enumerate(uniq)}
    plan = []                                     # w -> [(bank, olo, ohi, alo, ahi)]
    contrib = {}                                  # bank -> [w, ...]
    for w in range(N_WIN):
        e0 = (WIN * w) // N_NODES
        t_lo = N_NODES * e0
        t_hi = min(t_lo + 4 * N_NODES, TOK)
        ent = []
        for b in range(t_lo // SLICE, (t_hi - 1) // SLICE + 1):
            lo = max(t_lo, SLICE * b)
            hi = min(t_hi, SLICE * (b + 1))
            ent.append((b, lo - SLICE * b, hi - SLICE * b, lo - t_lo, hi - t_lo))
            contrib.setdefault(b, []).append(w)
        plan.append(ent)
    return phis, uniq, pidx, plan, contrib


def _build_nc():
    import concourse.bacc as bacc
    import concourse.mybir as mybir
    from concourse.tile import TileContext

    f32 = mybir.dt.float32
    f16 = mybir.dt.float16
    Alu = mybir.AluOpType
    Relu = mybir.ActivationFunctionType.Relu

    nc = bacc.Bacc(trn_type="TRN2")

    phis, uniq, pidx, plan, contrib = _phase_plan()
    n_ph = len(uniq)
    APH0 = 518                    # a2ph block starts here in the blob

    # blob16 cols: [0:384] wgT (3 layers x 128), [384:512] wd1T,
    # [512:518] wd2T, [518:518+13*208] A_phi phase matrices
    qT_d = nc.dram_tensor("qT", [FEAT, TOK], f16, kind="ExternalInput")
    winT_d = nc.dram_tensor("winT", [FEAT, HID], f16, kind="ExternalInput")
    blob_d = nc.dram_tensor("blob16", [HID, APH0 + n_ph * 208], f16,
                            kind="ExternalInput")
    # bias cols: 0 b_in, 1..3 b_gnn, 4 b_d1
    bias_d = nc.dram_tensor("biases", [HID, 5], f32, kind="ExternalInput")
    ver_d = nc.dram_tensor("vertag", [1, _KVER], f32, kind="ExternalInput")
    y_d = nc.dram_tensor("yTm", [TOK, FEAT], f16, kind="ExternalOutput")
    if _DEBUG:
        dbg0_d = nc.dram_tensor("dbgX0", [HID, TOK], f16, kind="ExternalOutput")
        dbg1_d = nc.dram_tensor("dbgX1", [HID, TOK], f16, kind="ExternalOutput")

    # greedy DVE/ACT balance: assign each PSUM-exit op to the engine with
    # the lower projected busy time (cost-model rates incl. fixed overheads)
    busy = {"A": 0.0, "D": 0.0}

    def next_eng(cols):
        ca = cols * 0.8333 + 215.0
        cd = cols * 1.0417 + 140.0
        if busy["A"] + ca / 2 <= busy["D"] + cd / 2:
            busy["A"] += ca
            return "A"
        busy["D"] += cd
        return "D"

    def opa(out, psum, bias_ap, zero):
        # out = relu(psum + bias) on DVE or ACT
        if next_eng(out.shape[-1]) == "A":
            nc.scalar.activation(out, psum, Relu, bias=bias_ap)
        else:
            nc.vector.scalar_tensor_tensor(
                out, psum, bias_ap, zero[:, :out.shape[-1]],
                op0=Alu.add, op1=Alu.max,
            )

    def copy(out, psum):
        if next_eng(out.shape[-1]) == "A":
            nc.scalar.copy(out, psum)
        else:
            nc.vector.tensor_copy(out, psum)

    with TileContext(nc) as tc:
        with (
            tc.sbuf_pool(name="const", bufs=1) as cp,
            tc.sbuf_pool(name="work", bufs=4) as wp,
            tc.psum_pool(name="ps", bufs=2) as pp,
        ):
            blob = cp.tile_from(blob_d[:, :])
            winT = cp.tile_from(winT_d[:, :])
            biases = cp.tile_from(bias_d[:, :])
            vtag = cp.tile([1, _KVER], f32)
            nc.sync.dma_start(vtag, ver_d[:, :])
            zero_s = cp.tile([128, 2 * SLICE], f16)
            nc.vector.memset(zero_s, 0.0)
            # warmup: trigger the one-time ACT function-table load while the
            # qT input DMA is still in flight
            warm = cp.tile([1, _KVER], f32)
            nc.scalar.activation(warm, vtag, Relu)

            wd1T = blob[:, 384:512]
            wd2T = blob[:, 512:518]
            bin_s = biases[:, 0:1]
            bd1_s = biases[:, 4:5]

            qT = cp.tile([FEAT, TOK], f16)
            for c in range(4):
                nc.sync.dma_start(
                    qT[:, c * (TOK // 4):(c + 1) * (TOK // 4)],
                    qT_d[:, c * (TOK // 4):(c + 1) * (TOK // 4)],
                )

            X = cp.tile([128, TOK], f16)

            # -------- encoder: X = relu(W_in @ q^T + b_in) ----------------
            enc_pend = []
            for t in range(N_SLICES // 2 + 1):
                if t < N_SLICES // 2:
                    if t % 2:
                        pa = pp.tile([128, SLICE], f32, tag="pc", bufs=4,
                                     name="encps1")
                        pb_ = pp.tile([128, SLICE], f32, tag="pc", bufs=4,
                                      name="encps2")
                        parts = [pa, pb_]
                    else:
                        enc_ps = pp.tile([128, 2 * SLICE], f32, tag="pb",
                                         bufs=2, name="encps")
                        parts = [enc_ps[:, :SLICE], enc_ps[:, SLICE:]]
                    for q in range(2):
                        s_ = 2 * t + q
                        nc.tensor.matmul(
                            parts[q], winT,
                            qT[:, s_ * SLICE:(s_ + 1) * SLICE],
                            start=True, stop=True,
                        )
                    if t % 2:
                        enc_pend.append((t, parts))
                    else:
                        enc_pend.append((t, [enc_ps]))
                # one-tile emission delay keeps the engines from reaching
                # an opA before its matmuls have retired
                while enc_pend and (enc_pend[0][0] < t or t == N_SLICES // 2):
                    tt, prts = enc_pend.pop(0)
                    if len(prts) == 2:
                        for q in range(2):
                            s_ = 2 * tt + q
                            opa(X[:, s_ * SLICE:(s_ + 1) * SLICE], prts[q],
                                bin_s, zero_s)
                    else:
                        opa(X[:, tt * 2 * SLICE:(tt + 1) * 2 * SLICE],
                            prts[0], bin_s, zero_s)

            if _DEBUG:
                nc.sync.dma_start(dbg0_d[:, :], X)

            # -------- GNN layers: x += relu(A (x W_l^T) + b_l) -----------
            # software-pipelined: the transpose+W matmuls for tile T are
            # emitted before the node mix of tile T-1 so the in-order PE
            # queue never stalls on the PSUM->SBUF copy
            for l in range(N_LAYERS):
                wgT_l = blob[:, l * 128:(l + 1) * 128]
                bg_l = biases[:, 1 + l:2 + l]
                ht_tiles = {}
                agg_open = {}
                done_banks = []
                r8 = None
                for T in range(N_SLICES // 2 + 1):       # 26 2-bank tiles
                    if T < N_SLICES // 2:
                        ht_ps = pp.tile([128, 2 * SLICE], f32, tag="pb",
                                        bufs=2, name="htps")
                        for i in range(8):
                            w = 8 * T + i
                            nc.tensor.matmul(
                                ht_ps[:, i * WIN:(i + 1) * WIN],
                                X[:, w * WIN:(w + 1) * WIN],
                                wgT_l,
                                start=True, stop=True,
                            )
                        ht_tiles[T] = ht_ps
                    if T == 0:
                        continue
                    TT = T - 1
                    hts = wp.tile([128, 2 * SLICE], f16, bufs=6)
                    copy(hts, ht_tiles.pop(TT))

                    # node mix: moving = A_phi slices, accumulating into
                    # 1-bank agg tiles with start/stop groups; enc/d1 share
                    # the pc tag so agg tiles use their own tag "pa"? no --
                    # pc is 2-bank bufs=2 for enc/d1, aggs use tag "pa"
                    for i in range(8):
                        w = 8 * TT + i
                        aph = blob[:, APH0 + pidx[phis[w]] * 208:
                                      APH0 + (pidx[phis[w]] + 1) * 208]
                        for (b, olo, ohi, alo, ahi) in plan[w]:
                            if b not in agg_open:
                                agg_open[b] = pp.tile(
                                    [128, SLICE], f32, tag="pc", bufs=4,
                                    name="aggps",
                                )
                            nc.tensor.matmul(
                                agg_open[b][:, olo:ohi],
                                hts[:, i * WIN:(i + 1) * WIN],
                                aph[:, alo:ahi],
                                start=(w == contrib[b][0]),
                                stop=(w == contrib[b][-1]),
                            )
                            if w == contrib[b][-1]:
                                done_banks.append(b)

                    # banks complete by the PREVIOUS tile are relu+bias'd
                    # now (emission delay keeps the engine queues from
                    # reaching an opA before its matmuls), then one
                    # accumulating SWDGE DMA per RB banks (4KB runs)
                    flush = (T == N_SLICES // 2)
                    while done_banks and (flush or len(done_banks) > 2):
                        b = done_banks.pop(0)
                        if b % RB == 0:
                            r8 = wp.tile([128, RB * SLICE], f16, bufs=3,
                                         name="r8")
                        opa(r8[:, (b % RB) * SLICE:(b % RB + 1) * SLICE],
                            agg_open.pop(b), bg_l, zero_s)
                        if b % RB == RB - 1:
                            nc.gpsimd.dma_start(
                                X[:, (b - RB + 1) * SLICE:(b + 1) * SLICE],
                                r8, accum_op=Alu.add,
                            )
                    assert len(agg_open) <= 4, (T, sorted(agg_open))
                if _DEBUG and l == 0:
                    nc.sync.dma_start(dbg1_d[:, :], X)

            # -------- decoder --------------------------------------------
            # d1 slices are batch-agnostic; d2 chunks (104 tokens) then
            # slice the persistent d1s tile at pair boundaries. d2 work is
            # emitted interleaved with d1 (per 8-chunk m-group, as soon as
            # its d1s tokens are written) so the tiny d2 matmuls hide under
            # d1's vector work instead of forming a serial tail
            d1s = cp.tile([128, TOK], f16)
            y16 = cp.tile([PAIR_T, (TOK // PAIR_T) * FEAT], f16)
            g_emitted = 0

            def emit_d2_upto(covered):
                nonlocal g_emitted
                while g_emitted < TOK // (8 * PAIR_T) and \
                        (g_emitted + 1) * 8 * PAIR_T <= covered:
                    g = g_emitted
                    y_ps = pp.tile([PAIR_T, 8 * FEAT], f32, tag="pc",
                                   bufs=4, name="yps")
                    for p in range(8):
                        c = g * 8 + p
                        nc.tensor.matmul(
                            y_ps[:, p * FEAT:(p + 1) * FEAT],
                            d1s[:, c * PAIR_T:(c + 1) * PAIR_T],
                            wd2T,
                            start=True, stop=True,
                        )
                    copy(y16[:, g * 8 * FEAT:(g + 1) * 8 * FEAT], y_ps)
                    if (g + 1) % 8 == 0:
                        k = g // 8
                        dst = y_d[k * YCHUNKS * PAIR_T:
                                  (k + 1) * YCHUNKS * PAIR_T, :]
                        nc.sync.dma_start(
                            dst.rearrange("(c t) f -> t c f", t=PAIR_T),
                            y16[:, k * YCHUNKS * FEAT:
                                   (k + 1) * YCHUNKS * FEAT].rearrange(
                                "t (c f) -> t c f", f=FEAT),
                        )
                    g_emitted += 1

            d1_pend = []
            for t in range(N_SLICES // 2 + 1):
                if t < N_SLICES // 2:
                    if t % 2:
                        pa = pp.tile([128, SLICE], f32, tag="pc", bufs=4,
                                     name="d1ps1")
                        pb_ = pp.tile([128, SLICE], f32, tag="pc", bufs=4,
                                      name="d1ps2")
                        parts = [pa, pb_]
                    else:
                        d1_ps = pp.tile([128, 2 * SLICE], f32, tag="pb",
                                        bufs=2, name="d1ps")
                        parts = [d1_ps[:, :SLICE], d1_ps[:, SLICE:]]
                    for q in range(2):
                        s_ = 2 * t + q
                        nc.tensor.matmul(
                            parts[q], wd1T,
                            X[:, s_ * SLICE:(s_ + 1) * SLICE],
                            start=True, stop=True,
                        )
                    if t % 2:
                        d1_pend.append((t, parts))
                    else:
                        d1_pend.append((t, [d1_ps]))
                while d1_pend and (d1_pend[0][0] < t or t == N_SLICES // 2):
                    tt, prts = d1_pend.pop(0)
                    if len(prts) == 2:
                        for q in range(2):
                            s_ = 2 * tt + q
                            opa(d1s[:, s_ * SLICE:(s_ + 1) * SLICE], prts[q],
                                bd1_s, zero_s)
                    else:
                        opa(d1s[:, tt * 2 * SLICE:(tt + 1) * 2 * SLICE],
                            prts[0], bd1_s, zero_s)
                    emit_d2_upto(2 * SLICE * tt)

            emit_d2_upto(TOK)

    nc.finalize()
    return nc


def _host_prep(inputs):
    q = np.asarray(inputs["q_current"], np.float32).reshape(BATCH, N_NODES, FEAT)
    W_in = np.asarray(inputs["W_in"], np.float32)
    b_in = np.asarray(inputs["b_in"], np.float32)
    W_gnn = np.asarray(inputs["W_gnn"], np.float32)
    b_gnn = np.asarray(inputs["b_gnn"], np.float32)
    W_d1 = np.asarray(inputs["W_d1"], np.float32)
    b_d1 = np.asarray(inputs["b_d1"], np.float32)
    W_d2 = np.asarray(inputs["W_d2"], np.float32)
    edge = np.asarray(inputs["edge_index"]).astype(np.int64)

    # dense normalized adjacency (PyG GCNConv default w/ self-loops)
    loops = np.arange(N_NODES, dtype=np.int64)
    src = np.concatenate([edge[0], loops])
    dst = np.concatenate([edge[1], loops])
    deg = np.zeros(N_NODES, np.float32)
    np.add.at(deg, dst, 1.0)
    dinv = 1.0 / np.sqrt(np.maximum(deg, 1e-12))
    A = np.zeros((N_NODES, N_NODES), np.float32)
    np.add.at(A, (dst, src), dinv[src] * dinv[dst])

    # phase matrices: A_phi[t, 52*k + u] = A[u, node(phi + t)] for the
    # batch element k = (phi + t) // 52 the source token belongs to
    phis, uniq, pidx, plan, contrib = _phase_plan()
    n_ph = len(uniq)
    aph = np.zeros((n_ph, WIN, 4 * N_NODES), np.float32)
    for pi, phi in enumerate(uniq):
        for t in range(WIN):
            k = (phi + t) // N_NODES
            n = (phi + t) % N_NODES
            aph[pi, t, N_NODES * k:N_NODES * (k + 1)] = A[:, n]

    APH0 = 518
    blob = np.zeros((HID, APH0 + n_ph * 208), np.float32)
    blob[:, 0:384] = W_gnn.transpose(2, 0, 1).reshape(HID, N_LAYERS * HID)
    blob[:, 384:512] = W_d1.T
    blob[:, 512:518] = W_d2.T
    blob[:WIN, APH0:] = aph.transpose(1, 0, 2).reshape(WIN, n_ph * 208)

    biases = np.zeros((HID, 5), np.float32)
    biases[:, 0] = b_in
    biases[:, 1:4] = b_gnn.T
    biases[:, 4] = b_d1

    const = {
        "winT": np.ascontiguousarray(W_in.T).astype(np.float16),
        "blob16": blob.astype(np.float16),
        "biases": np.ascontiguousarray(biases),
        "vertag": np.zeros((1, _KVER), np.float32),
    }

    # per-core feature-major input [6, TOK], fp16
    q_flat = q.reshape(N_CORES, B_C * N_NODES, FEAT)
    qTs = [
        np.ascontiguousarray(q_flat[c].T).astype(np.float16)
        for c in range(N_CORES)
    ]
    return const, qTs


def kernel(**inputs):
    const, qTs = _host_prep(inputs)

    if "nc" not in _CACHE:
        _CACHE["nc"] = _build_nc()
    nc = _CACHE["nc"]

    from concourse.bass_utils import run_bass_kernel_spmd

    in_maps = [dict(const, qT=qTs[c]) for c in range(N_CORES)]
    res = run_bass_kernel_spmd(nc, in_maps, core_ids=list(range(N_CORES)))
    global _LAST_EXEC_NS
    _LAST_EXEC_NS = res.exec_time_ns
    if res.instructions_and_trace is not None:
        _LAST_TRACE.append(res.instructions_and_trace[1])

    b_d2 = np.asarray(inputs["b_d2"], np.float32)
    outs = []
    for c in range(N_CORES):
        yTm = res.results[c]["yTm"]  # [TOK, 6] fp16
        outs.append(np.asarray(yTm, np.float32) + b_d2)
    y = np.concatenate(outs, axis=0)  # [BATCH*52, 6]
    return np.ascontiguousarray(y).reshape(BATCH, N_FIL, N_SUB, FEAT)


# revision 43
# speedup vs baseline: 1.2690x; 1.0028x over previous
"""Batched GCN (microtubule dynamics model) on 8 Trainium2 NeuronCores.

Math: the reference's gather/scale/scatter message passing over a fixed
52-node graph is a dense linear operator on the node axis:
    agg[b] = A @ h[b],  A[i, j] = sum over edges (j->i, incl self-loops)
                                   of dinv[src] * dinv[dst]
and A commutes with the shared linear layer, so each GNN layer is
    x += relu((A @ x) @ W_l^T + b_l),  batched over B.

Device strategy (pure data parallel, 512 batch elems / core):
  - activations live in SBUF as [128 hid partitions, 26624 token cols]
    (token = b*52 + node), fp16 on-chip, fp32 PSUM for matmuls
  - per layer the token axis is processed in FULL 128-token windows
    (no per-batch-pair padding): fused transpose+W matmuls (stationary
    = 128-token X window, moving = W_l^T) fill one PSUM bank per 4
    windows -> ONE PSUM->SBUF copy per 512 tokens -> the node mix uses
    13 precomputed phase matrices A_phi[src 128, 4*52] (phi = window
    offset mod 52) as the moving operand, accumulating into per-bank
    agg PSUM tiles with start/stop groups -> relu+bias per full 512-col
    bank -> one accumulating SWDGE DMA per 4 banks does the residual
    (the SWDGE accumulate path corrupts beyond 4KB contiguous runs, so
    runs are kept at exactly 4KB)
  - PSUM-reading vector work cannot run on Pool (BIR: GPSIMD cannot
    access PSUM), so copies and relu+bias are greedily balanced over
    DVE and ACT by projected busy time; Pool only triggers SWDGE
  - decoder: d1 like the encoder; d2 exploits that matmul cost ~ moving
    free size: stationary = relu(d1) 104-token chunk, moving = W_d2^T
    (6 cols) -> token-major y in PSUM, copied once per 64 chunks and
    DMA'd to HBM as [TOK, 6] fp16; b_d2 is added during host unshard
"""

import numpy as np

N_FIL, N_SUB = 13, 4
N_NODES = N_FIL * N_SUB          # 52
FEAT = 6
HID = 128
N_LAYERS = 3
BATCH = 4096
N_CORES = 8
B_C = BATCH // N_CORES           # 512 batch elems per core
TOK = B_C * N_NODES              # 26624 token columns per core
PAIR_T = 2 * N_NODES             # 104 tokens per decoder d2 chunk
WIN = 128                        # tokens per transpose window
N_WIN = TOK // WIN               # 208 windows per layer
SLICE = 512                      # psum bank (fp32 cols)
N_SLICES = TOK // SLICE          # 52 banks of tokens
RB = 4                           # agg banks per residual accum DMA (4KB)
YCHUNKS = 64                     # d2 token-chunks per psum y tile

_CACHE = {}
_LAST_EXEC_NS = None
_LAST_TRACE = []
_DEBUG = False   # adds dbgX0/dbgX1 DRAM outputs (X after enc / after layer 1)
_KVER = 16       # bump on every semantic change: the execution service caches
                 # compiled NEFFs by program signature, and a changing input
                 # shape is the only reliable cache-buster


def _phase_plan():
    """Per-window node-mix plan against 512-token agg banks."""
    phis = [(WIN * w) % N_NODES for w in range(N_WIN)]
    uniq = sorted(set(phis))                      # 13 phases
    pidx = {p: i for i, p in enumerate(uniq)}
    plan = []                                     # w -> [(bank, olo, ohi, alo, ahi)]
    contrib = {}                                  # bank -> [w, ...]
    for w in range(N_WIN):
        e0 = (WIN * w) // N_NODES
        t_lo = N_NODES * e0
        t_hi = min(t_lo + 4 * N_NODES, TOK)
        ent = []
        for b in range(t_lo // SLICE, (t_hi - 1) // SLICE + 1):
            lo = max(t_lo, SLICE * b)
            hi = min(t_hi, SLICE * (b + 1))
            ent.append((b, lo - SLICE * b, hi - SLICE * b, lo - t_lo, hi - t_lo))
            contrib.setdefault(b, []).append(w)
        plan.append(ent)
    return phis, uniq, pidx, plan, contrib


def _build_nc():
    import concourse.bacc as bacc
    import concourse.mybir as mybir
    from concourse.tile import TileContext

    f32 = mybir.dt.float32
    f16 = mybir.dt.float16
    Alu = mybir.AluOpType
    Relu = mybir.ActivationFunctionType.Relu

    nc = bacc.Bacc(trn_type="TRN2")

    phis, uniq, pidx, plan, contrib = _phase_plan()
    n_ph = len(uniq)
    APH0 = 518                    # a2ph block starts here in the blob

    # blob16 cols: [0:384] wgT (3 layers x 128), [384:512] wd1T,
    # [512:518] wd2T, [518:518+13*208] A_phi phase matrices
    qT_d = nc.dram_tensor("qT", [FEAT, TOK], f16, kind="ExternalInput")
    winT_d = nc.dram_tensor("winT", [FEAT, HID], f16, kind="ExternalInput")
    blob_d = nc.dram_tensor("blob16", [HID, APH0 + n_ph * 208], f16,
                            kind="ExternalInput")
    # bias cols: 0 b_in, 1..3 b_gnn, 4 b_d1
    bias_d = nc.dram_tensor("biases", [HID, 5], f32, kind="ExternalInput")
    ver_d = nc.dram_tensor("vertag", [1, _KVER], f32, kind="ExternalInput")
    y_d = nc.dram_tensor("yTm", [TOK, FEAT], f16, kind="ExternalOutput")
    if _DEBUG:
        dbg0_d = nc.dram_tensor("dbgX0", [HID, TOK], f16, kind="ExternalOutput")
        dbg1_d = nc.dram_tensor("dbgX1", [HID, TOK], f16, kind="ExternalOutput")

    # greedy DVE/ACT balance: assign each PSUM-exit op to the engine with
    # the lower projected busy time (cost-model rates incl. fixed overheads)
    busy = {"A": 0.0, "D": 0.0}

    def next_eng(cols):
        ca = cols * 0.8333 + 215.0
        cd = cols * 1.0417 + 140.0
        if busy["A"] + ca / 2 <= busy["D"] + cd / 2:
            busy["A"] += ca
            return "A"
        busy["D"] += cd
        return "D"

    def opa(out, psum, bias_ap, zero):
        # out = relu(psum + bias) on DVE or ACT
        if next_eng(out.shape[-1]) == "A":
            nc.scalar.activation(out, psum, Relu, bias=bias_ap)
        else:
            nc.vector.scalar_tensor_tensor(
                out, psum, bias_ap, zero[:, :out.shape[-1]],
                op0=Alu.add, op1=Alu.max,
            )

    def copy(out, psum):
        if next_eng(out.shape[-1]) == "A":
            nc.scalar.copy(out, psum)
        else:
            nc.vector.tensor_copy(out, psum)

    with TileContext(nc) as tc:
        with (
            tc.sbuf_pool(name="const", bufs=1) as cp,
            tc.sbuf_pool(name="work", bufs=4) as wp,
            tc.psum_pool(name="ps", bufs=2) as pp,
        ):
            blob = cp.tile_from(blob_d[:, :])
            winT = cp.tile_from(winT_d[:, :])
            biases = cp.tile_from(bias_d[:, :])
            vtag = cp.tile([1, _KVER], f32)
            nc.sync.dma_start(vtag, ver_d[:, :])
            zero_s = cp.tile([128, 2 * SLICE], f16)
            nc.vector.memset(zero_s, 0.0)
            # warmup: trigger the one-time ACT function-table load while the
            # qT input DMA is still in flight
            warm = cp.tile([1, _KVER], f32)
            nc.scalar.activation(warm, vtag, Relu)

            wd1T = blob[:, 384:512]
            wd2T = blob[:, 512:518]
            bin_s = biases[:, 0:1]
            bd1_s = biases[:, 4:5]

            qT = cp.tile([FEAT, TOK], f16)
            for c in range(4):
                nc.sync.dma_start(
                    qT[:, c * (TOK // 4):(c + 1) * (TOK // 4)],
                    qT_d[:, c * (TOK // 4):(c + 1) * (TOK // 4)],
                )

            X = cp.tile([128, TOK], f16)

            # -------- encoder: X = relu(W_in @ q^T + b_in) ----------------
            enc_pend = []
            for t in range(N_SLICES // 2 + 1):
                if t < N_SLICES // 2:
                    if t % 2:
                        pa = pp.tile([128, SLICE], f32, tag="pc", bufs=4,
                                     name="encps1")
                        pb_ = pp.tile([128, SLICE], f32, tag="pc", bufs=4,
                                      name="encps2")
                        parts = [pa, pb_]
                    else:
                        enc_ps = pp.tile([128, 2 * SLICE], f32, tag="pb",
                                         bufs=2, name="encps")
                        parts = [enc_ps[:, :SLICE], enc_ps[:, SLICE:]]
                    for q in range(2):
                        s_ = 2 * t + q
                        nc.tensor.matmul(
                            parts[q], winT,
                            qT[:, s_ * SLICE:(s_ + 1) * SLICE],
                            start=True, stop=True,
                        )
                    if t % 2:
                        enc_pend.append((t, parts))
                    else:
                        enc_pend.append((t, [enc_ps]))
                # one-tile emission delay keeps the engines from reaching
                # an opA before its matmuls have retired
                while enc_pend and (enc_pend[0][0] < t or t == N_SLICES // 2):
                    tt, prts = enc_pend.pop(0)
                    if len(prts) == 2:
                        for q in range(2):
                            s_ = 2 * tt + q
                            opa(X[:, s_ * SLICE:(s_ + 1) * SLICE], prts[q],
                                bin_s, zero_s)
                    else:
                        opa(X[:, tt * 2 * SLICE:(tt + 1) * 2 * SLICE],
                            prts[0], bin_s, zero_s)

            if _DEBUG:
                nc.sync.dma_start(dbg0_d[:, :], X)

            # -------- GNN layers: x += relu(A (x W_l^T) + b_l) -----------
            # software-pipelined: the transpose+W matmuls for tile T are
            # emitted before the node mix of tile T-1 so the in-order PE
            # queue never stalls on the PSUM->SBUF copy
            for l in range(N_LAYERS):
                wgT_l = blob[:, l * 128:(l + 1) * 128]
                bg_l = biases[:, 1 + l:2 + l]
                ht_tiles = {}
                agg_open = {}
                done_banks = []
                r8 = None
                for T in range(N_SLICES // 2 + 1):       # 26 2-bank tiles
                    if T < N_SLICES // 2:
                        ht_ps = pp.tile([128, 2 * SLICE], f32, tag="pb",
                                        bufs=2, name="htps")
                        for i in range(8):
                            w = 8 * T + i
                            nc.tensor.matmul(
                                ht_ps[:, i * WIN:(i + 1) * WIN],
                                X[:, w * WIN:(w + 1) * WIN],
                                wgT_l,
                                start=True, stop=True,
                            )
                        ht_tiles[T] = ht_ps
                    if T == 0:
                        continue
                    TT = T - 1
                    hts = wp.tile([128, 2 * SLICE], f16, bufs=6)
                    copy(hts, ht_tiles.pop(TT))

                    # node mix: moving = A_phi slices, accumulating into
                    # 1-bank agg tiles with start/stop groups; enc/d1 share
                    # the pc tag so agg tiles use their own tag "pa"? no --
                    # pc is 2-bank bufs=2 for enc/d1, aggs use tag "pa"
                    for i in range(8):
                        w = 8 * TT + i
                        aph = blob[:, APH0 + pidx[phis[w]] * 208:
                                      APH0 + (pidx[phis[w]] + 1) * 208]
                        for (b, olo, ohi, alo, ahi) in plan[w]:
                            if b not in agg_open:
                                agg_open[b] = pp.tile(
                                    [128, SLICE], f32, tag="pc", bufs=4,
                                    name="aggps",
                                )
                            nc.tensor.matmul(
                                agg_open[b][:, olo:ohi],
                                hts[:, i * WIN:(i + 1) * WIN],
                                aph[:, alo:ahi],
                                start=(w == contrib[b][0]),
                                stop=(w == contrib[b][-1]),
                            )
                            if w == contrib[b][-1]:
                                done_banks.append(b)

                    # banks complete by the PREVIOUS tile are relu+bias'd
                    # now (emission delay keeps the engine queues from
                    # reaching an opA before its matmuls), then one
                    # accumulating SWDGE DMA per RB banks (4KB runs)
                    flush = (T == N_SLICES // 2)
                    while done_banks and (flush or len(done_banks) > 2):
                        b = done_banks.pop(0)
                        if b % RB == 0:
                            r8 = wp.tile([128, RB * SLICE], f16, bufs=3,
                                         name="r8")
                        opa(r8[:, (b % RB) * SLICE:(b % RB + 1) * SLICE],
                            agg_open.pop(b), bg_l, zero_s)
                        if b % RB == RB - 1:
                            nc.gpsimd.dma_start(
                                X[:, (b - RB + 1) * SLICE:(b + 1) * SLICE],
                                r8, accum_op=Alu.add,
                            )
                    assert len(agg_open) <= 4, (T, sorted(agg_open))
                if _DEBUG and l == 0:
                    nc.sync.dma_start(dbg1_d[:, :], X)

            # -------- decoder --------------------------------------------
            # d1 slices are batch-agnostic; d2 chunks (104 tokens) then
            # slice the persistent d1s tile at pair boundaries. d2 work is
            # emitted interleaved with d1 (per 8-chunk m-group, as soon as
            # its d1s tokens are written) so the tiny d2 matmuls hide under
            # d1's vector work instead of forming a serial tail
            d1s = cp.tile([128, TOK], f16)
            y16 = cp.tile([PAIR_T, (TOK // PAIR_T) * FEAT], f16)
            g_emitted = 0

            y_ps_pend = [None]

            def emit_d2_upto(covered):
                nonlocal g_emitted
                while g_emitted < TOK // (8 * PAIR_T) and \
                        (g_emitted + 1) * 8 * PAIR_T <= covered:
                    g = g_emitted
                    if g % 2 == 0:
                        y_ps_pend[0] = pp.tile([PAIR_T, 16 * FEAT], f32,
                                               tag="pc", bufs=4, name="yps")
                    y_ps = y_ps_pend[0]
                    half = (g % 2) * 8 * FEAT
                    for p in range(8):
                        c = g * 8 + p
                        nc.tensor.matmul(
                            y_ps[:, half + p * FEAT:half + (p + 1) * FEAT],
                            d1s[:, c * PAIR_T:(c + 1) * PAIR_T],
                            wd2T,
                            start=True, stop=True,
                        )
                    if g % 2 == 1:
                        copy(y16[:, (g - 1) * 8 * FEAT:(g + 1) * 8 * FEAT],
                             y_ps)
                    if (g + 1) % 8 == 0:
                        k = g // 8
                        dst = y_d[k * YCHUNKS * PAIR_T:
                                  (k + 1) * YCHUNKS * PAIR_T, :]
                        nc.sync.dma_start(
                            dst.rearrange("(c t) f -> t c f", t=PAIR_T),
                            y16[:, k * YCHUNKS * FEAT:
                                   (k + 1) * YCHUNKS * FEAT].rearrange(
                                "t (c f) -> t c f", f=FEAT),
                        )
                    g_emitted += 1

            d1_pend = []
            for t in range(N_SLICES // 2 + 1):
                if t < N_SLICES // 2:
                    if t % 2:
                        pa = pp.tile([128, SLICE], f32, tag="pc", bufs=4,
                                     name="d1ps1")
                        pb_ = pp.tile([128, SLICE], f32, tag="pc", bufs=4,
                                      name="d1ps2")
                        parts = [pa, pb_]
                    else:
                        d1_ps = pp.tile([128, 2 * SLICE], f32, tag="pb",
                                        bufs=2, name="d1ps")
                        parts = [d1_ps[:, :SLICE], d1_ps[:, SLICE:]]
                    for q in range(2):
                        s_ = 2 * t + q
                        nc.tensor.matmul(
                            parts[q], wd1T,
                            X[:, s_ * SLICE:(s_ + 1) * SLICE],
                            start=True, stop=True,
                        )
                    if t % 2:
                        d1_pend.append((t, parts))
                    else:
                        d1_pend.append((t, [d1_ps]))
                while d1_pend and (d1_pend[0][0] < t or t == N_SLICES // 2):
                    tt, prts = d1_pend.pop(0)
                    if len(prts) == 2:
                        for q in range(2):
                            s_ = 2 * tt + q
                            opa(d1s[:, s_ * SLICE:(s_ + 1) * SLICE], prts[q],
                                bd1_s, zero_s)
                    else:
                        opa(d1s[:, tt * 2 * SLICE:(tt + 1) * 2 * SLICE],
                            prts[0], bd1_s, zero_s)
                    emit_d2_upto(2 * SLICE * (tt + 1))

            emit_d2_upto(TOK)

    nc.finalize()
    return nc


def _host_prep(inputs):
    q = np.asarray(inputs["q_current"], np.float32).reshape(BATCH, N_NODES, FEAT)
    W_in = np.asarray(inputs["W_in"], np.float32)
    b_in = np.asarray(inputs["b_in"], np.float32)
    W_gnn = np.asarray(inputs["W_gnn"], np.float32)
    b_gnn = np.asarray(inputs["b_gnn"], np.float32)
    W_d1 = np.asarray(inputs["W_d1"], np.float32)
    b_d1 = np.asarray(inputs["b_d1"], np.float32)
    W_d2 = np.asarray(inputs["W_d2"], np.float32)
    edge = np.asarray(inputs["edge_index"]).astype(np.int64)

    # dense normalized adjacency (PyG GCNConv default w/ self-loops)
    loops = np.arange(N_NODES, dtype=np.int64)
    src = np.concatenate([edge[0], loops])
    dst = np.concatenate([edge[1], loops])
    deg = np.zeros(N_NODES, np.float32)
    np.add.at(deg, dst, 1.0)
    dinv = 1.0 / np.sqrt(np.maximum(deg, 1e-12))
    A = np.zeros((N_NODES, N_NODES), np.float32)
    np.add.at(A, (dst, src), dinv[src] * dinv[dst])

    # phase matrices: A_phi[t, 52*k + u] = A[u, node(phi + t)] for the
    # batch element k = (phi + t) // 52 the source token belongs to
    phis, uniq, pidx, plan, contrib = _phase_plan()
    n_ph = len(uniq)
    aph = np.zeros((n_ph, WIN, 4 * N_NODES), np.float32)
    for pi, phi in enumerate(uniq):
        for t in range(WIN):
            k = (phi + t) // N_NODES
            n = (phi + t) % N_NODES
            aph[pi, t, N_NODES * k:N_NODES * (k + 1)] = A[:, n]

    APH0 = 518
    blob = np.zeros((HID, APH0 + n_ph * 208), np.float32)
    blob[:, 0:384] = W_gnn.transpose(2, 0, 1).reshape(HID, N_LAYERS * HID)
    blob[:, 384:512] = W_d1.T
    blob[:, 512:518] = W_d2.T
    blob[:WIN, APH0:] = aph.transpose(1, 0, 2).reshape(WIN, n_ph * 208)

    biases = np.zeros((HID, 5), np.float32)
    biases[:, 0] = b_in
    biases[:, 1:4] = b_gnn.T
    biases[:, 4] = b_d1

    const = {
        "winT": np.ascontiguousarray(W_in.T).astype(np.float16),
        "blob16": blob.astype(np.float16),
        "biases": np.ascontiguousarray(biases),
        "vertag": np.zeros((1, _KVER), np.float32),
    }

    # per-core feature-major input [6, TOK], fp16
    q_flat = q.reshape(N_CORES, B_C * N_NODES, FEAT)
    qTs = [
        np.ascontiguousarray(q_flat[c].T).astype(np.float16)
        for c in range(N_CORES)
    ]
    return const, qTs


def kernel(**inputs):
    const, qTs = _host_prep(inputs)

    if "nc" not in _CACHE:
        _CACHE["nc"] = _build_nc()
    nc = _CACHE["nc"]

    from concourse.bass_utils import run_bass_kernel_spmd

    in_maps = [dict(const, qT=qTs[c]) for c in range(N_CORES)]
    res = run_bass_kernel_spmd(nc, in_maps, core_ids=list(range(N_CORES)))
    global _LAST_EXEC_NS
    _LAST_EXEC_NS = res.exec_time_ns
    if res.instructions_and_trace is not None:
        _LAST_TRACE.append(res.instructions_and_trace[1])

    b_d2 = np.asarray(inputs["b_d2"], np.float32)
    outs = []
    for c in range(N_CORES):
        yTm = res.results[c]["yTm"]  # [TOK, 6] fp16
        outs.append(np.asarray(yTm, np.float32) + b_d2)
    y = np.concatenate(outs, axis=0)  # [BATCH*52, 6]
    return np.ascontiguousarray(y).reshape(BATCH, N_FIL, N_SUB, FEAT)


# revision 48
# speedup vs baseline: 1.2803x; 1.0089x over previous
"""Batched GCN (microtubule dynamics model) on 8 Trainium2 NeuronCores.

Math: the reference's gather/scale/scatter message passing over a fixed
52-node graph is a dense linear operator on the node axis:
    agg[b] = A @ h[b],  A[i, j] = sum over edges (j->i, incl self-loops)
                                   of dinv[src] * dinv[dst]
and A commutes with the shared linear layer, so each GNN layer is
    x += relu((A @ x) @ W_l^T + b_l),  batched over B.

Device strategy (pure data parallel, 512 batch elems / core):
  - activations live in SBUF as [128 hid partitions, 26624 token cols]
    (token = b*52 + node), fp16 on-chip, fp32 PSUM for matmuls
  - per layer the token axis is processed in FULL 128-token windows
    (no per-batch-pair padding): fused transpose+W matmuls (stationary
    = 128-token X window, moving = W_l^T) fill one PSUM bank per 4
    windows -> ONE PSUM->SBUF copy per 512 tokens -> the node mix uses
    13 precomputed phase matrices A_phi[src 128, 4*52] (phi = window
    offset mod 52) as the moving operand, accumulating into per-bank
    agg PSUM tiles with start/stop groups -> relu+bias per full 512-col
    bank -> one accumulating SWDGE DMA per 4 banks does the residual
    (the SWDGE accumulate path corrupts beyond 4KB contiguous runs, so
    runs are kept at exactly 4KB)
  - PSUM-reading vector work cannot run on Pool (BIR: GPSIMD cannot
    access PSUM), so copies and relu+bias are greedily balanced over
    DVE and ACT by projected busy time; Pool only triggers SWDGE
  - decoder: d1 like the encoder; d2 exploits that matmul cost ~ moving
    free size: stationary = relu(d1) 104-token chunk, moving = W_d2^T
    (6 cols) -> token-major y in PSUM, copied once per 64 chunks and
    DMA'd to HBM as [TOK, 6] fp16; b_d2 is added during host unshard
"""

import numpy as np

N_FIL, N_SUB = 13, 4
N_NODES = N_FIL * N_SUB          # 52
FEAT = 6
HID = 128
N_LAYERS = 3
BATCH = 4096
N_CORES = 8
B_C = BATCH // N_CORES           # 512 batch elems per core
TOK = B_C * N_NODES              # 26624 token columns per core
PAIR_T = 2 * N_NODES             # 104 tokens per decoder d2 chunk
WIN = 128                        # tokens per transpose window
N_WIN = TOK // WIN               # 208 windows per layer
SLICE = 512                      # psum bank (fp32 cols)
N_SLICES = TOK // SLICE          # 52 banks of tokens
RB = 4                           # agg banks per residual accum DMA (4KB)
YCHUNKS = 64                     # d2 token-chunks per psum y tile

_CACHE = {}
_LAST_EXEC_NS = None
_LAST_TRACE = []
_DEBUG = False   # adds dbgX0/dbgX1 DRAM outputs (X after enc / after layer 1)
_KVER = 17       # bump on every semantic change: the execution service caches
                 # compiled NEFFs by program signature, and a changing input
                 # shape is the only reliable cache-buster


def _phase_plan():
    """Per-window node-mix plan against 512-token agg banks."""
    phis = [(WIN * w) % N_NODES for w in range(N_WIN)]
    uniq = sorted(set(phis))                      # 13 phases
    pidx = {p: i for i, p in enumerate(uniq)}
    plan = []                                     # w -> [(bank, olo, ohi, alo, ahi)]
    contrib = {}                                  # bank -> [w, ...]
    for w in range(N_WIN):
        e0 = (WIN * w) // N_NODES
        t_lo = N_NODES * e0
        t_hi = min(t_lo + 4 * N_NODES, TOK)
        ent = []
        for b in range(t_lo // SLICE, (t_hi - 1) // SLICE + 1):
            lo = max(t_lo, SLICE * b)
            hi = min(t_hi, SLICE * (b + 1))
            ent.append((b, lo - SLICE * b, hi - SLICE * b, lo - t_lo, hi - t_lo))
            contrib.setdefault(b, []).append(w)
        plan.append(ent)
    return phis, uniq, pidx, plan, contrib


def _build_nc():
    import concourse.bacc as bacc
    import concourse.mybir as mybir
    from concourse.tile import TileContext

    f32 = mybir.dt.float32
    f16 = mybir.dt.float16
    Alu = mybir.AluOpType
    Relu = mybir.ActivationFunctionType.Relu

    nc = bacc.Bacc(trn_type="TRN2")

    phis, uniq, pidx, plan, contrib = _phase_plan()
    n_ph = len(uniq)
    APH0 = 518                    # a2ph block starts here in the blob

    # blob16 cols: [0:384] wgT (3 layers x 128), [384:512] wd1T,
    # [512:518] wd2T, [518:518+13*208] A_phi phase matrices
    qT_d = nc.dram_tensor("qT", [FEAT, TOK], f16, kind="ExternalInput")
    winT_d = nc.dram_tensor("winT", [FEAT, HID], f16, kind="ExternalInput")
    blob_d = nc.dram_tensor("blob16", [HID, APH0 + n_ph * 208], f16,
                            kind="ExternalInput")
    # bias cols: 0 b_in, 1..3 b_gnn, 4 b_d1
    bias_d = nc.dram_tensor("biases", [HID, 5], f32, kind="ExternalInput")
    ver_d = nc.dram_tensor("vertag", [1, _KVER], f32, kind="ExternalInput")
    y_d = nc.dram_tensor("yTm", [TOK, FEAT], f16, kind="ExternalOutput")
    if _DEBUG:
        dbg0_d = nc.dram_tensor("dbgX0", [HID, TOK], f16, kind="ExternalOutput")
        dbg1_d = nc.dram_tensor("dbgX1", [HID, TOK], f16, kind="ExternalOutput")

    # greedy DVE/ACT balance: assign each PSUM-exit op to the engine with
    # the lower projected busy time (cost-model rates incl. fixed overheads)
    busy = {"A": 0.0, "D": 0.0}

    def next_eng(cols):
        ca = cols * 0.8333 + 215.0
        cd = cols * 1.0417 + 140.0
        if busy["A"] + ca / 2 <= busy["D"] + cd / 2:
            busy["A"] += ca
            return "A"
        busy["D"] += cd
        return "D"

    def opa(out, psum, bias_ap, zero):
        # out = relu(psum + bias) on DVE or ACT
        if next_eng(out.shape[-1]) == "A":
            nc.scalar.activation(out, psum, Relu, bias=bias_ap)
        else:
            nc.vector.scalar_tensor_tensor(
                out, psum, bias_ap, zero[:, :out.shape[-1]],
                op0=Alu.add, op1=Alu.max,
            )

    def copy(out, psum):
        if next_eng(out.shape[-1]) == "A":
            nc.scalar.copy(out, psum)
        else:
            nc.vector.tensor_copy(out, psum)

    with TileContext(nc) as tc:
        with (
            tc.sbuf_pool(name="const", bufs=1) as cp,
            tc.sbuf_pool(name="work", bufs=4) as wp,
            tc.psum_pool(name="ps", bufs=2) as pp,
        ):
            blob = cp.tile_from(blob_d[:, :])
            winT = cp.tile_from(winT_d[:, :])
            biases = cp.tile_from(bias_d[:, :])
            vtag = cp.tile([1, _KVER], f32)
            nc.sync.dma_start(vtag, ver_d[:, :])
            zero_s = cp.tile([128, 2 * SLICE], f16)
            nc.vector.memset(zero_s, 0.0)
            # warmup: trigger the one-time ACT function-table load while the
            # qT input DMA is still in flight
            warm = cp.tile([1, _KVER], f32)
            nc.scalar.activation(warm, vtag, Relu)

            wd1T = blob[:, 384:512]
            wd2T = blob[:, 512:518]
            bin_s = biases[:, 0:1]
            bd1_s = biases[:, 4:5]

            qT = cp.tile([FEAT, TOK], f16)
            for c in range(8):
                nc.sync.dma_start(
                    qT[:, c * (TOK // 8):(c + 1) * (TOK // 8)],
                    qT_d[:, c * (TOK // 8):(c + 1) * (TOK // 8)],
                )

            X = cp.tile([128, TOK], f16)

            # -------- encoder: X = relu(W_in @ q^T + b_in) ----------------
            enc_pend = []
            for t in range(N_SLICES // 2 + 1):
                if t < N_SLICES // 2:
                    if t % 2:
                        pa = pp.tile([128, SLICE], f32, tag="pc", bufs=4,
                                     name="encps1")
                        pb_ = pp.tile([128, SLICE], f32, tag="pc", bufs=4,
                                      name="encps2")
                        parts = [pa, pb_]
                    else:
                        enc_ps = pp.tile([128, 2 * SLICE], f32, tag="pb",
                                         bufs=2, name="encps")
                        parts = [enc_ps[:, :SLICE], enc_ps[:, SLICE:]]
                    for q in range(2):
                        s_ = 2 * t + q
                        nc.tensor.matmul(
                            parts[q], winT,
                            qT[:, s_ * SLICE:(s_ + 1) * SLICE],
                            start=True, stop=True,
                        )
                    if t % 2:
                        enc_pend.append((t, parts))
                    else:
                        enc_pend.append((t, [enc_ps]))
                # one-tile emission delay keeps the engines from reaching
                # an opA before its matmuls have retired
                while enc_pend and (enc_pend[0][0] < t - 1 or t == N_SLICES // 2):
                    tt, prts = enc_pend.pop(0)
                    if len(prts) == 2:
                        for q in range(2):
                            s_ = 2 * tt + q
                            opa(X[:, s_ * SLICE:(s_ + 1) * SLICE], prts[q],
                                bin_s, zero_s)
                    else:
                        opa(X[:, tt * 2 * SLICE:(tt + 1) * 2 * SLICE],
                            prts[0], bin_s, zero_s)

            if _DEBUG:
                nc.sync.dma_start(dbg0_d[:, :], X)

            # -------- GNN layers: x += relu(A (x W_l^T) + b_l) -----------
            # software-pipelined: the transpose+W matmuls for tile T are
            # emitted before the node mix of tile T-1 so the in-order PE
            # queue never stalls on the PSUM->SBUF copy
            for l in range(N_LAYERS):
                wgT_l = blob[:, l * 128:(l + 1) * 128]
                bg_l = biases[:, 1 + l:2 + l]
                ht_tiles = {}
                agg_open = {}
                done_banks = []
                r8 = None
                for T in range(N_SLICES // 2 + 1):       # 26 2-bank tiles
                    if T < N_SLICES // 2:
                        ht_ps = pp.tile([128, 2 * SLICE], f32, tag="pb",
                                        bufs=2, name="htps")
                        for i in range(8):
                            w = 8 * T + i
                            nc.tensor.matmul(
                                ht_ps[:, i * WIN:(i + 1) * WIN],
                                X[:, w * WIN:(w + 1) * WIN],
                                wgT_l,
                                start=True, stop=True,
                            )
                        ht_tiles[T] = ht_ps
                    if T == 0:
                        continue
                    TT = T - 1
                    hts = wp.tile([128, 2 * SLICE], f16, bufs=6)
                    copy(hts, ht_tiles.pop(TT))

                    # node mix: moving = A_phi slices, accumulating into
                    # 1-bank agg tiles with start/stop groups; enc/d1 share
                    # the pc tag so agg tiles use their own tag "pa"? no --
                    # pc is 2-bank bufs=2 for enc/d1, aggs use tag "pa"
                    for i in range(8):
                        w = 8 * TT + i
                        aph = blob[:, APH0 + pidx[phis[w]] * 208:
                                      APH0 + (pidx[phis[w]] + 1) * 208]
                        for (b, olo, ohi, alo, ahi) in plan[w]:
                            if b not in agg_open:
                                agg_open[b] = pp.tile(
                                    [128, SLICE], f32, tag="pc", bufs=4,
                                    name="aggps",
                                )
                            nc.tensor.matmul(
                                agg_open[b][:, olo:ohi],
                                hts[:, i * WIN:(i + 1) * WIN],
                                aph[:, alo:ahi],
                                start=(w == contrib[b][0]),
                                stop=(w == contrib[b][-1]),
                            )
                            if w == contrib[b][-1]:
                                done_banks.append(b)

                    # banks complete by the PREVIOUS tile are relu+bias'd
                    # now (emission delay keeps the engine queues from
                    # reaching an opA before its matmuls), then one
                    # accumulating SWDGE DMA per RB banks (4KB runs)
                    flush = (T == N_SLICES // 2)
                    while done_banks and (flush or len(done_banks) > 2):
                        b = done_banks.pop(0)
                        if b % RB == 0:
                            r8 = wp.tile([128, RB * SLICE], f16, bufs=3,
                                         name="r8")
                        opa(r8[:, (b % RB) * SLICE:(b % RB + 1) * SLICE],
                            agg_open.pop(b), bg_l, zero_s)
                        if b % RB == RB - 1:
                            nc.gpsimd.dma_start(
                                X[:, (b - RB + 1) * SLICE:(b + 1) * SLICE],
                                r8, accum_op=Alu.add,
                            )
                    assert len(agg_open) <= 4, (T, sorted(agg_open))
                if _DEBUG and l == 0:
                    nc.sync.dma_start(dbg1_d[:, :], X)

            # -------- decoder --------------------------------------------
            # d1 slices are batch-agnostic; d2 chunks (104 tokens) then
            # slice the persistent d1s tile at pair boundaries. d2 work is
            # emitted interleaved with d1 (per 8-chunk m-group, as soon as
            # its d1s tokens are written) so the tiny d2 matmuls hide under
            # d1's vector work instead of forming a serial tail
            d1s = cp.tile([128, TOK], f16)
            y16 = cp.tile([PAIR_T, (TOK // PAIR_T) * FEAT], f16)
            g_emitted = 0

            y_ps_pend = [None]

            def emit_d2_upto(covered):
                nonlocal g_emitted
                while g_emitted < TOK // (8 * PAIR_T) and \
                        (g_emitted + 1) * 8 * PAIR_T <= covered:
                    g = g_emitted
                    if g % 2 == 0:
                        y_ps_pend[0] = pp.tile([PAIR_T, 16 * FEAT], f32,
                                               tag="pc", bufs=4, name="yps")
                    y_ps = y_ps_pend[0]
                    half = (g % 2) * 8 * FEAT
                    for p in range(8):
                        c = g * 8 + p
                        nc.tensor.matmul(
                            y_ps[:, half + p * FEAT:half + (p + 1) * FEAT],
                            d1s[:, c * PAIR_T:(c + 1) * PAIR_T],
                            wd2T,
                            start=True, stop=True,
                        )
                    if g % 2 == 1:
                        copy(y16[:, (g - 1) * 8 * FEAT:(g + 1) * 8 * FEAT],
                             y_ps)
                    if (g + 1) % 8 == 0:
                        k = g // 8
                        dst = y_d[k * YCHUNKS * PAIR_T:
                                  (k + 1) * YCHUNKS * PAIR_T, :]
                        nc.sync.dma_start(
                            dst.rearrange("(c t) f -> t c f", t=PAIR_T),
                            y16[:, k * YCHUNKS * FEAT:
                                   (k + 1) * YCHUNKS * FEAT].rearrange(
                                "t (c f) -> t c f", f=FEAT),
                        )
                    g_emitted += 1

            d1_pend = []
            for t in range(N_SLICES // 2 + 1):
                if t < N_SLICES // 2:
                    if t % 2:
                        pa = pp.tile([128, SLICE], f32, tag="pc", bufs=4,
                                     name="d1ps1")
                        pb_ = pp.tile([128, SLICE], f32, tag="pc", bufs=4,
                                      name="d1ps2")
                        parts = [pa, pb_]
                    else:
                        d1_ps = pp.tile([128, 2 * SLICE], f32, tag="pb",
                                        bufs=2, name="d1ps")
                        parts = [d1_ps[:, :SLICE], d1_ps[:, SLICE:]]
                    for q in range(2):
                        s_ = 2 * t + q
                        nc.tensor.matmul(
                            parts[q], wd1T,
                            X[:, s_ * SLICE:(s_ + 1) * SLICE],
                            start=True, stop=True,
                        )
                    if t % 2:
                        d1_pend.append((t, parts))
                    else:
                        d1_pend.append((t, [d1_ps]))
                while d1_pend and (d1_pend[0][0] < t - 1 or t == N_SLICES // 2):
                    tt, prts = d1_pend.pop(0)
                    if len(prts) == 2:
                        for q in range(2):
                            s_ = 2 * tt + q
                            opa(d1s[:, s_ * SLICE:(s_ + 1) * SLICE], prts[q],
                                bd1_s, zero_s)
                    else:
                        opa(d1s[:, tt * 2 * SLICE:(tt + 1) * 2 * SLICE],
                            prts[0], bd1_s, zero_s)
                    emit_d2_upto(2 * SLICE * (tt + 1))

            emit_d2_upto(TOK)

    nc.finalize()
    return nc


def _host_prep(inputs):
    q = np.asarray(inputs["q_current"], np.float32).reshape(BATCH, N_NODES, FEAT)
    W_in = np.asarray(inputs["W_in"], np.float32)
    b_in = np.asarray(inputs["b_in"], np.float32)
    W_gnn = np.asarray(inputs["W_gnn"], np.float32)
    b_gnn = np.asarray(inputs["b_gnn"], np.float32)
    W_d1 = np.asarray(inputs["W_d1"], np.float32)
    b_d1 = np.asarray(inputs["b_d1"], np.float32)
    W_d2 = np.asarray(inputs["W_d2"], np.float32)
    edge = np.asarray(inputs["edge_index"]).astype(np.int64)

    # dense normalized adjacency (PyG GCNConv default w/ self-loops)
    loops = np.arange(N_NODES, dtype=np.int64)
    src = np.concatenate([edge[0], loops])
    dst = np.concatenate([edge[1], loops])
    deg = np.zeros(N_NODES, np.float32)
    np.add.at(deg, dst, 1.0)
    dinv = 1.0 / np.sqrt(np.maximum(deg, 1e-12))
    A = np.zeros((N_NODES, N_NODES), np.float32)
    np.add.at(A, (dst, src), dinv[src] * dinv[dst])

    # phase matrices: A_phi[t, 52*k + u] = A[u, node(phi + t)] for the
    # batch element k = (phi + t) // 52 the source token belongs to
    phis, uniq, pidx, plan, contrib = _phase_plan()
    n_ph = len(uniq)
    aph = np.zeros((n_ph, WIN, 4 * N_NODES), np.float32)
    for pi, phi in enumerate(uniq):
        for t in range(WIN):
            k = (phi + t) // N_NODES
            n = (phi + t) % N_NODES
            aph[pi, t, N_NODES * k:N_NODES * (k + 1)] = A[:, n]

    APH0 = 518
    blob = np.zeros((HID, APH0 + n_ph * 208), np.float32)
    blob[:, 0:384] = W_gnn.transpose(2, 0, 1).reshape(HID, N_LAYERS * HID)
    blob[:, 384:512] = W_d1.T
    blob[:, 512:518] = W_d2.T
    blob[:WIN, APH0:] = aph.transpose(1, 0, 2).reshape(WIN, n_ph * 208)

    biases = np.zeros((HID, 5), np.float32)
    biases[:, 0] = b_in
    biases[:, 1:4] = b_gnn.T
    biases[:, 4] = b_d1

    const = {
        "winT": np.ascontiguousarray(W_in.T).astype(np.float16),
        "blob16": blob.astype(np.float16),
        "biases": np.ascontiguousarray(biases),
        "vertag": np.zeros((1, _KVER), np.float32),
    }

    # per-core feature-major input [6, TOK], fp16
    q_flat = q.reshape(N_CORES, B_C * N_NODES, FEAT)
    qTs = [
        np.ascontiguousarray(q_flat[c].T).astype(np.float16)
        for c in range(N_CORES)
    ]
    return const, qTs


def kernel(**inputs):
    const, qTs = _host_prep(inputs)

    if "nc" not in _CACHE:
        _CACHE["nc"] = _build_nc()
    nc = _CACHE["nc"]

    from concourse.bass_utils import run_bass_kernel_spmd

    in_maps = [dict(const, qT=qTs[c]) for c in range(N_CORES)]
    res = run_bass_kernel_spmd(nc, in_maps, core_ids=list(range(N_CORES)))
    global _LAST_EXEC_NS
    _LAST_EXEC_NS = res.exec_time_ns
    if res.instructions_and_trace is not None:
        _LAST_TRACE.append(res.instructions_and_trace[1])

    b_d2 = np.asarray(inputs["b_d2"], np.float32)
    outs = []
    for c in range(N_CORES):
        yTm = res.results[c]["yTm"]  # [TOK, 6] fp16
        outs.append(np.asarray(yTm, np.float32) + b_d2)
    y = np.concatenate(outs, axis=0)  # [BATCH*52, 6]
    return np.ascontiguousarray(y).reshape(BATCH, N_FIL, N_SUB, FEAT)


# revision 49
# speedup vs baseline: 1.2848x; 1.0034x over previous
"""Batched GCN (microtubule dynamics model) on 8 Trainium2 NeuronCores.

Math: the reference's gather/scale/scatter message passing over a fixed
52-node graph is a dense linear operator on the node axis:
    agg[b] = A @ h[b],  A[i, j] = sum over edges (j->i, incl self-loops)
                                   of dinv[src] * dinv[dst]
and A commutes with the shared linear layer, so each GNN layer is
    x += relu((A @ x) @ W_l^T + b_l),  batched over B.

Device strategy (pure data parallel, 512 batch elems / core):
  - activations live in SBUF as [128 hid partitions, 26624 token cols]
    (token = b*52 + node), fp16 on-chip, fp32 PSUM for matmuls
  - per layer the token axis is processed in FULL 128-token windows
    (no per-batch-pair padding): fused transpose+W matmuls (stationary
    = 128-token X window, moving = W_l^T) fill one PSUM bank per 4
    windows -> ONE PSUM->SBUF copy per 512 tokens -> the node mix uses
    13 precomputed phase matrices A_phi[src 128, 4*52] (phi = window
    offset mod 52) as the moving operand, accumulating into per-bank
    agg PSUM tiles with start/stop groups -> relu+bias per full 512-col
    bank -> one accumulating SWDGE DMA per 4 banks does the residual
    (the SWDGE accumulate path corrupts beyond 4KB contiguous runs, so
    runs are kept at exactly 4KB)
  - PSUM-reading vector work cannot run on Pool (BIR: GPSIMD cannot
    access PSUM), so copies and relu+bias are greedily balanced over
    DVE and ACT by projected busy time; Pool only triggers SWDGE
  - decoder: d1 like the encoder; d2 exploits that matmul cost ~ moving
    free size: stationary = relu(d1) 104-token chunk, moving = W_d2^T
    (6 cols) -> token-major y in PSUM, copied once per 64 chunks and
    DMA'd to HBM as [TOK, 6] fp16; b_d2 is added during host unshard
"""

import numpy as np

N_FIL, N_SUB = 13, 4
N_NODES = N_FIL * N_SUB          # 52
FEAT = 6
HID = 128
N_LAYERS = 3
BATCH = 4096
N_CORES = 8
B_C = BATCH // N_CORES           # 512 batch elems per core
TOK = B_C * N_NODES              # 26624 token columns per core
PAIR_T = 2 * N_NODES             # 104 tokens per decoder d2 chunk
WIN = 128                        # tokens per transpose window
N_WIN = TOK // WIN               # 208 windows per layer
SLICE = 512                      # psum bank (fp32 cols)
N_SLICES = TOK // SLICE          # 52 banks of tokens
RB = 4                           # agg banks per residual accum DMA (4KB)
YCHUNKS = 64                     # d2 token-chunks per psum y tile

_CACHE = {}
_LAST_EXEC_NS = None
_LAST_TRACE = []
_DEBUG = False   # adds dbgX0/dbgX1 DRAM outputs (X after enc / after layer 1)
_KVER = 17       # bump on every semantic change: the execution service caches
                 # compiled NEFFs by program signature, and a changing input
                 # shape is the only reliable cache-buster


def _phase_plan():
    """Per-window node-mix plan against 512-token agg banks."""
    phis = [(WIN * w) % N_NODES for w in range(N_WIN)]
    uniq = sorted(set(phis))                      # 13 phases
    pidx = {p: i for i, p in enumerate(uniq)}
    plan = []                                     # w -> [(bank, olo, ohi, alo, ahi)]
    contrib = {}                                  # bank -> [w, ...]
    for w in range(N_WIN):
        e0 = (WIN * w) // N_NODES
        t_lo = N_NODES * e0
        t_hi = min(t_lo + 4 * N_NODES, TOK)
        ent = []
        for b in range(t_lo // SLICE, (t_hi - 1) // SLICE + 1):
            lo = max(t_lo, SLICE * b)
            hi = min(t_hi, SLICE * (b + 1))
            ent.append((b, lo - SLICE * b, hi - SLICE * b, lo - t_lo, hi - t_lo))
            contrib.setdefault(b, []).append(w)
        plan.append(ent)
    return phis, uniq, pidx, plan, contrib


def _build_nc():
    import concourse.bacc as bacc
    import concourse.mybir as mybir
    from concourse.tile import TileContext

    f32 = mybir.dt.float32
    f16 = mybir.dt.float16
    Alu = mybir.AluOpType
    Relu = mybir.ActivationFunctionType.Relu

    nc = bacc.Bacc(trn_type="TRN2")

    phis, uniq, pidx, plan, contrib = _phase_plan()
    n_ph = len(uniq)
    APH0 = 518                    # a2ph block starts here in the blob

    # blob16 cols: [0:384] wgT (3 layers x 128), [384:512] wd1T,
    # [512:518] wd2T, [518:518+13*208] A_phi phase matrices
    qT_d = nc.dram_tensor("qT", [FEAT, TOK], f16, kind="ExternalInput")
    winT_d = nc.dram_tensor("winT", [FEAT, HID], f16, kind="ExternalInput")
    blob_d = nc.dram_tensor("blob16", [HID, APH0 + n_ph * 208], f16,
                            kind="ExternalInput")
    # bias cols: 0 b_in, 1..3 b_gnn, 4 b_d1
    bias_d = nc.dram_tensor("biases", [HID, 5], f32, kind="ExternalInput")
    ver_d = nc.dram_tensor("vertag", [1, _KVER], f32, kind="ExternalInput")
    y_d = nc.dram_tensor("yTm", [TOK, FEAT], f16, kind="ExternalOutput")
    if _DEBUG:
        dbg0_d = nc.dram_tensor("dbgX0", [HID, TOK], f16, kind="ExternalOutput")
        dbg1_d = nc.dram_tensor("dbgX1", [HID, TOK], f16, kind="ExternalOutput")

    # greedy DVE/ACT balance: assign each PSUM-exit op to the engine with
    # the lower projected busy time (cost-model rates incl. fixed overheads)
    busy = {"A": 0.0, "D": 0.0}

    def next_eng(cols):
        ca = cols * 0.8333 + 215.0
        cd = cols * 1.0417 + 140.0
        if busy["A"] + ca / 2 <= busy["D"] + cd / 2:
            busy["A"] += ca
            return "A"
        busy["D"] += cd
        return "D"

    def opa(out, psum, bias_ap, zero):
        # out = relu(psum + bias) on DVE or ACT
        if next_eng(out.shape[-1]) == "A":
            nc.scalar.activation(out, psum, Relu, bias=bias_ap)
        else:
            nc.vector.scalar_tensor_tensor(
                out, psum, bias_ap, zero[:, :out.shape[-1]],
                op0=Alu.add, op1=Alu.max,
            )

    def copy(out, psum):
        if next_eng(out.shape[-1]) == "A":
            nc.scalar.copy(out, psum)
        else:
            nc.vector.tensor_copy(out, psum)

    with TileContext(nc) as tc:
        with (
            tc.sbuf_pool(name="const", bufs=1) as cp,
            tc.sbuf_pool(name="work", bufs=4) as wp,
            tc.psum_pool(name="ps", bufs=2) as pp,
        ):
            blob = cp.tile_from(blob_d[:, :])
            winT = cp.tile_from(winT_d[:, :])
            biases = cp.tile_from(bias_d[:, :])
            vtag = cp.tile([1, _KVER], f32)
            nc.sync.dma_start(vtag, ver_d[:, :])
            zero_s = cp.tile([128, 2 * SLICE], f16)
            nc.vector.memset(zero_s, 0.0)
            # warmup: trigger the one-time ACT function-table load while the
            # qT input DMA is still in flight
            warm = cp.tile([1, _KVER], f32)
            nc.scalar.activation(warm, vtag, Relu)

            wd1T = blob[:, 384:512]
            wd2T = blob[:, 512:518]
            bin_s = biases[:, 0:1]
            bd1_s = biases[:, 4:5]

            qT = cp.tile([FEAT, TOK], f16)
            for c in range(8):
                nc.sync.dma_start(
                    qT[:, c * (TOK // 8):(c + 1) * (TOK // 8)],
                    qT_d[:, c * (TOK // 8):(c + 1) * (TOK // 8)],
                )

            X = cp.tile([128, TOK], f16)

            # -------- encoder: X = relu(W_in @ q^T + b_in) ----------------
            enc_pend = []
            for t in range(N_SLICES // 2 + 1):
                if t < N_SLICES // 2:
                    if t % 2:
                        pa = pp.tile([128, SLICE], f32, tag="pc", bufs=4,
                                     name="encps1")
                        pb_ = pp.tile([128, SLICE], f32, tag="pc", bufs=4,
                                      name="encps2")
                        parts = [pa, pb_]
                    else:
                        enc_ps = pp.tile([128, 2 * SLICE], f32, tag="pb",
                                         bufs=2, name="encps")
                        parts = [enc_ps[:, :SLICE], enc_ps[:, SLICE:]]
                    for q in range(2):
                        s_ = 2 * t + q
                        nc.tensor.matmul(
                            parts[q], winT,
                            qT[:, s_ * SLICE:(s_ + 1) * SLICE],
                            start=True, stop=True,
                        )
                    if t % 2:
                        enc_pend.append((t, parts))
                    else:
                        enc_pend.append((t, [enc_ps]))
                # one-tile emission delay keeps the engines from reaching
                # an opA before its matmuls have retired
                while enc_pend and (enc_pend[0][0] < t - 2 or t == N_SLICES // 2):
                    tt, prts = enc_pend.pop(0)
                    if len(prts) == 2:
                        for q in range(2):
                            s_ = 2 * tt + q
                            opa(X[:, s_ * SLICE:(s_ + 1) * SLICE], prts[q],
                                bin_s, zero_s)
                    else:
                        opa(X[:, tt * 2 * SLICE:(tt + 1) * 2 * SLICE],
                            prts[0], bin_s, zero_s)

            if _DEBUG:
                nc.sync.dma_start(dbg0_d[:, :], X)

            # -------- GNN layers: x += relu(A (x W_l^T) + b_l) -----------
            # software-pipelined: the transpose+W matmuls for tile T are
            # emitted before the node mix of tile T-1 so the in-order PE
            # queue never stalls on the PSUM->SBUF copy
            for l in range(N_LAYERS):
                wgT_l = blob[:, l * 128:(l + 1) * 128]
                bg_l = biases[:, 1 + l:2 + l]
                ht_tiles = {}
                agg_open = {}
                done_banks = []
                r8 = None
                for T in range(N_SLICES // 2 + 1):       # 26 2-bank tiles
                    if T < N_SLICES // 2:
                        ht_ps = pp.tile([128, 2 * SLICE], f32, tag="pb",
                                        bufs=2, name="htps")
                        for i in range(8):
                            w = 8 * T + i
                            nc.tensor.matmul(
                                ht_ps[:, i * WIN:(i + 1) * WIN],
                                X[:, w * WIN:(w + 1) * WIN],
                                wgT_l,
                                start=True, stop=True,
                            )
                        ht_tiles[T] = ht_ps
                    if T == 0:
                        continue
                    TT = T - 1
                    hts = wp.tile([128, 2 * SLICE], f16, bufs=6)
                    copy(hts, ht_tiles.pop(TT))

                    # node mix: moving = A_phi slices, accumulating into
                    # 1-bank agg tiles with start/stop groups; enc/d1 share
                    # the pc tag so agg tiles use their own tag "pa"? no --
                    # pc is 2-bank bufs=2 for enc/d1, aggs use tag "pa"
                    for i in range(8):
                        w = 8 * TT + i
                        aph = blob[:, APH0 + pidx[phis[w]] * 208:
                                      APH0 + (pidx[phis[w]] + 1) * 208]
                        for (b, olo, ohi, alo, ahi) in plan[w]:
                            if b not in agg_open:
                                agg_open[b] = pp.tile(
                                    [128, SLICE], f32, tag="pc", bufs=4,
                                    name="aggps",
                                )
                            nc.tensor.matmul(
                                agg_open[b][:, olo:ohi],
                                hts[:, i * WIN:(i + 1) * WIN],
                                aph[:, alo:ahi],
                                start=(w == contrib[b][0]),
                                stop=(w == contrib[b][-1]),
                            )
                            if w == contrib[b][-1]:
                                done_banks.append(b)

                    # banks complete by the PREVIOUS tile are relu+bias'd
                    # now (emission delay keeps the engine queues from
                    # reaching an opA before its matmuls), then one
                    # accumulating SWDGE DMA per RB banks (4KB runs)
                    flush = (T == N_SLICES // 2)
                    while done_banks and (flush or len(done_banks) > 2):
                        b = done_banks.pop(0)
                        if b % RB == 0:
                            r8 = wp.tile([128, RB * SLICE], f16, bufs=3,
                                         name="r8")
                        opa(r8[:, (b % RB) * SLICE:(b % RB + 1) * SLICE],
                            agg_open.pop(b), bg_l, zero_s)
                        if b % RB == RB - 1:
                            nc.gpsimd.dma_start(
                                X[:, (b - RB + 1) * SLICE:(b + 1) * SLICE],
                                r8, accum_op=Alu.add,
                            )
                    assert len(agg_open) <= 4, (T, sorted(agg_open))
                if _DEBUG and l == 0:
                    nc.sync.dma_start(dbg1_d[:, :], X)

            # -------- decoder --------------------------------------------
            # d1 slices are batch-agnostic; d2 chunks (104 tokens) then
            # slice the persistent d1s tile at pair boundaries. d2 work is
            # emitted interleaved with d1 (per 8-chunk m-group, as soon as
            # its d1s tokens are written) so the tiny d2 matmuls hide under
            # d1's vector work instead of forming a serial tail
            d1s = cp.tile([128, TOK], f16)
            y16 = cp.tile([PAIR_T, (TOK // PAIR_T) * FEAT], f16)
            g_emitted = 0

            y_ps_pend = [None]

            def emit_d2_upto(covered):
                nonlocal g_emitted
                while g_emitted < TOK // (8 * PAIR_T) and \
                        (g_emitted + 1) * 8 * PAIR_T <= covered:
                    g = g_emitted
                    if g % 2 == 0:
                        y_ps_pend[0] = pp.tile([PAIR_T, 16 * FEAT], f32,
                                               tag="pc", bufs=4, name="yps")
                    y_ps = y_ps_pend[0]
                    half = (g % 2) * 8 * FEAT
                    for p in range(8):
                        c = g * 8 + p
                        nc.tensor.matmul(
                            y_ps[:, half + p * FEAT:half + (p + 1) * FEAT],
                            d1s[:, c * PAIR_T:(c + 1) * PAIR_T],
                            wd2T,
                            start=True, stop=True,
                        )
                    if g % 2 == 1:
                        copy(y16[:, (g - 1) * 8 * FEAT:(g + 1) * 8 * FEAT],
                             y_ps)
                    if (g + 1) % 8 == 0:
                        k = g // 8
                        dst = y_d[k * YCHUNKS * PAIR_T:
                                  (k + 1) * YCHUNKS * PAIR_T, :]
                        nc.sync.dma_start(
                            dst.rearrange("(c t) f -> t c f", t=PAIR_T),
                            y16[:, k * YCHUNKS * FEAT:
                                   (k + 1) * YCHUNKS * FEAT].rearrange(
                                "t (c f) -> t c f", f=FEAT),
                        )
                    g_emitted += 1

            d1_pend = []
            for t in range(N_SLICES // 2 + 1):
                if t < N_SLICES // 2:
                    if t % 2:
                        pa = pp.tile([128, SLICE], f32, tag="pc", bufs=4,
                                     name="d1ps1")
                        pb_ = pp.tile([128, SLICE], f32, tag="pc", bufs=4,
                                      name="d1ps2")
                        parts = [pa, pb_]
                    else:
                        d1_ps = pp.tile([128, 2 * SLICE], f32, tag="pb",
                                        bufs=2, name="d1ps")
                        parts = [d1_ps[:, :SLICE], d1_ps[:, SLICE:]]
                    for q in range(2):
                        s_ = 2 * t + q
                        nc.tensor.matmul(
                            parts[q], wd1T,
                            X[:, s_ * SLICE:(s_ + 1) * SLICE],
                            start=True, stop=True,
                        )
                    if t % 2:
                        d1_pend.append((t, parts))
                    else:
                        d1_pend.append((t, [d1_ps]))
                while d1_pend and (d1_pend[0][0] < t - 2 or t == N_SLICES // 2):
                    tt, prts = d1_pend.pop(0)
                    if len(prts) == 2:
                        for q in range(2):
                            s_ = 2 * tt + q
                            opa(d1s[:, s_ * SLICE:(s_ + 1) * SLICE], prts[q],
                                bd1_s, zero_s)
                    else:
                        opa(d1s[:, tt * 2 * SLICE:(tt + 1) * 2 * SLICE],
                            prts[0], bd1_s, zero_s)
                    emit_d2_upto(2 * SLICE * (tt + 1))

            emit_d2_upto(TOK)

    nc.finalize()
    return nc


def _host_prep(inputs):
    q = np.asarray(inputs["q_current"], np.float32).reshape(BATCH, N_NODES, FEAT)
    W_in = np.asarray(inputs["W_in"], np.float32)
    b_in = np.asarray(inputs["b_in"], np.float32)
    W_gnn = np.asarray(inputs["W_gnn"], np.float32)
    b_gnn = np.asarray(inputs["b_gnn"], np.float32)
    W_d1 = np.asarray(inputs["W_d1"], np.float32)
    b_d1 = np.asarray(inputs["b_d1"], np.float32)
    W_d2 = np.asarray(inputs["W_d2"], np.float32)
    edge = np.asarray(inputs["edge_index"]).astype(np.int64)

    # dense normalized adjacency (PyG GCNConv default w/ self-loops)
    loops = np.arange(N_NODES, dtype=np.int64)
    src = np.concatenate([edge[0], loops])
    dst = np.concatenate([edge[1], loops])
    deg = np.zeros(N_NODES, np.float32)
    np.add.at(deg, dst, 1.0)
    dinv = 1.0 / np.sqrt(np.maximum(deg, 1e-12))
    A = np.zeros((N_NODES, N_NODES), np.float32)
    np.add.at(A, (dst, src), dinv[src] * dinv[dst])

    # phase matrices: A_phi[t, 52*k + u] = A[u, node(phi + t)] for the
    # batch element k = (phi + t) // 52 the source token belongs to
    phis, uniq, pidx, plan, contrib = _phase_plan()
    n_ph = len(uniq)
    aph = np.zeros((n_ph, WIN, 4 * N_NODES), np.float32)
    for pi, phi in enumerate(uniq):
        for t in range(WIN):
            k = (phi + t) // N_NODES
            n = (phi + t) % N_NODES
            aph[pi, t, N_NODES * k:N_NODES * (k + 1)] = A[:, n]

    APH0 = 518
    blob = np.zeros((HID, APH0 + n_ph * 208), np.float32)
    blob[:, 0:384] = W_gnn.transpose(2, 0, 1).reshape(HID, N_LAYERS * HID)
    blob[:, 384:512] = W_d1.T
    blob[:, 512:518] = W_d2.T
    blob[:WIN, APH0:] = aph.transpose(1, 0, 2).reshape(WIN, n_ph * 208)

    biases = np.zeros((HID, 5), np.float32)
    biases[:, 0] = b_in
    biases[:, 1:4] = b_gnn.T
    biases[:, 4] = b_d1

    const = {
        "winT": np.ascontiguousarray(W_in.T).astype(np.float16),
        "blob16": blob.astype(np.float16),
        "biases": np.ascontiguousarray(biases),
        "vertag": np.zeros((1, _KVER), np.float32),
    }

    # per-core feature-major input [6, TOK], fp16
    q_flat = q.reshape(N_CORES, B_C * N_NODES, FEAT)
    qTs = [
        np.ascontiguousarray(q_flat[c].T).astype(np.float16)
        for c in range(N_CORES)
    ]
    return const, qTs


def kernel(**inputs):
    const, qTs = _host_prep(inputs)

    if "nc" not in _CACHE:
        _CACHE["nc"] = _build_nc()
    nc = _CACHE["nc"]

    from concourse.bass_utils import run_bass_kernel_spmd

    in_maps = [dict(const, qT=qTs[c]) for c in range(N_CORES)]
    res = run_bass_kernel_spmd(nc, in_maps, core_ids=list(range(N_CORES)))
    global _LAST_EXEC_NS
    _LAST_EXEC_NS = res.exec_time_ns
    if res.instructions_and_trace is not None:
        _LAST_TRACE.append(res.instructions_and_trace[1])

    b_d2 = np.asarray(inputs["b_d2"], np.float32)
    outs = []
    for c in range(N_CORES):
        yTm = res.results[c]["yTm"]  # [TOK, 6] fp16
        outs.append(np.asarray(yTm, np.float32) + b_d2)
    y = np.concatenate(outs, axis=0)  # [BATCH*52, 6]
    return np.ascontiguousarray(y).reshape(BATCH, N_FIL, N_SUB, FEAT)
